# revision 19
# baseline (speedup 1.0000x reference)
"""GCGRU cell (order-2 graph diffusion GRU) Trainium2 Bass kernel, v2.

Strategy: data-parallel over batch (B=16 -> 2 batches per core x 8 cores).
The dominant cost in v1 was streaming the 32MB fp16 adjacency from HBM four
times per core (DMA 99% busy). v2 keeps the whole adjacency RESIDENT in SBUF
as fp8 (x4096 pre-scale keeps the row-normalized values out of e4m3's
denormal range), loaded once (~16MB), and runs all four diffusion passes as
fp8 DoubleRow matmuls (2 packed contraction rows/cycle). Diffused features
are small contributors to the output (the graph averages 4000 nodes), so fp8
error lands ~1e-4 relative; order-k features carry power-of-2 scales folded
into the PSUM-evacuation copies and undone by host-side weight pre-scaling.

Layouts per core: activations node-major fp8 [128p x (chunk, col)] for
diffusion; gate/candidate convs run fp16 from per-band staging tiles
(PE transposes for diffused features, XBAR DMA-transpose from DRAM for the
raw [x;h] features). Gate/candidate nonlinearities on ACT, elementwise on
DVE, combine fused into the last diffusion's band loop.
"""

import numpy as np
import ml_dtypes

import concourse.bass as bass
from concourse import bacc
import concourse.mybir as mybir
import concourse.tile as tile
from concourse.bass_utils import run_bass_kernel_spmd

# problem constants
B, D_IN, D_H, NN = 16, 32, 64, 4000
NCORES = 8
B_LOC = B // NCORES          # batches per core
C = D_IN + D_H               # 96 channels into each gate conv
BC = B_LOC * C               # node-major column count (b-major: [b0 c96 | b1 c96])
BH = B_LOC * D_H             # stacked batch-hidden rows (128)
NP = 4096                    # contraction node dim padded to 32 chunks
CHUNK = 128
NCH = NP // CHUNK            # 32 contraction chunks
NPR = NCH // 2               # 16 DoubleRow chunk pairs
NBAND = 8                    # output-node bands: 7x512 + 416 (= 4000, no pad)
BW = [512] * 7 + [416]
BOFF = [512 * g for g in range(NBAND)]
AOFF = [NCH * 512 * g for g in range(NBAND)]   # at_d col offset per band

F8 = mybir.dt.float8e4
F16 = mybir.dt.float16
F32 = mybir.dt.float32
DR = mybir.MatmulPerfMode.DoubleRow
E4NP = ml_dtypes.float8_e4m3

# fp8 scale chain: adjacency carries x4096 (2^12).
#   z1T carries x32   -> evac scale 32/4096
#   z2T carries x512  -> evac scale 512/(4096*32)
#   zc1 carries x64   -> evac scale 64/4096
#   zc2 stage x512    -> evac scale 512/(4096*64)
S_Z1E, S_Z2E = 2.0 ** -7, 2.0 ** -8
S_C1E, S_C2E = 2.0 ** -6, 2.0 ** -9
# matching host-side weight descales: gate W1 /32, W2 /512; cand x-part
# W1 /32, W2 /512; cand rh-part W1 /64, W2 /512.


def _mlist(g):
    """(offset, width) of the 128-wide m-chunks inside band g."""
    w = BW[g]
    out = []
    mo = 0
    while mo < w:
        out.append((mo, min(CHUNK, w - mo)))
        mo += CHUNK
    return out


def build_program():
    nc = bacc.Bacc("TRN2", target_bir_lowering=False, debug=False)

    at_d = nc.dram_tensor("at", [CHUNK, NCH * NN], F8, kind="ExternalInput").ap()
    zt_d = nc.dram_tensor("zt", [CHUNK, NCH * BC], F8, kind="ExternalInput").ap()
    # node-major [x;h] fp16, padded to 128 cols/batch for XBAR dma transpose
    zn_d = nc.dram_tensor("zn", [NP, B_LOC, CHUNK], F16, kind="ExternalInput").ap()
    h_d = nc.dram_tensor("h", [BH, NN], F16, kind="ExternalInput").ap()
    wf_d = nc.dram_tensor("wf", [3, C, D_H], F16, kind="ExternalInput").ap()
    wu_d = nc.dram_tensor("wu", [3, C, D_H], F16, kind="ExternalInput").ap()
    wcx_d = nc.dram_tensor("wcx", [3, D_IN, D_H], F16, kind="ExternalInput").ap()
    wcrh_d = nc.dram_tensor("wcrh", [3, D_H, D_H], F16, kind="ExternalInput").ap()
    bf_d = nc.dram_tensor("bf", [BH, 1], F32, kind="ExternalInput").ap()
    bu_d = nc.dram_tensor("bu", [BH, 1], F32, kind="ExternalInput").ap()
    bc_d = nc.dram_tensor("bcb", [BH, 1], F32, kind="ExternalInput").ap()
    id16_d = nc.dram_tensor("id16", [CHUNK, CHUNK], F16, kind="ExternalInput").ap()
    id8_d = nc.dram_tensor("id8", [CHUNK, CHUNK], F8, kind="ExternalInput").ap()
    out_d = nc.dram_tensor("out", [B_LOC, D_H, NN], F32, kind="ExternalOutput").ap()

    with tile.TileContext(nc) as tc:
        _body(tc, locals())
    nc.compile()
    return nc


def _body(tc, aps):
    nc = tc.nc
    at_d, zt_d, zn_d, h_d = aps["at_d"], aps["zt_d"], aps["zn_d"], aps["h_d"]
    wf_d, wu_d, wcx_d, wcrh_d = (
        aps["wf_d"], aps["wu_d"], aps["wcx_d"], aps["wcrh_d"])
    bf_d, bu_d, bc_d = aps["bf_d"], aps["bu_d"], aps["bc_d"]
    id16_d, id8_d, out_d = aps["id16_d"], aps["id8_d"], aps["out_d"]

    SIG = mybir.ActivationFunctionType.Sigmoid
    TANH = mybir.ActivationFunctionType.Tanh
    COPY = mybir.ActivationFunctionType.Copy

    with (
        tc.tile_pool(name="const", bufs=1) as cpool,
        tc.tile_pool(name="amat", bufs=1) as apool,       # resident adjacency
        tc.tile_pool(name="nm8", bufs=2) as nmpool,       # rotating node-major fp8
        tc.tile_pool(name="perst", bufs=1) as ppool,
        tc.tile_pool(name="stageA", bufs=2) as sApool,    # conv feature stages
        tc.tile_pool(name="stageB", bufs=2) as sBpool,    # wide f16 stages
        tc.tile_pool(name="stageC", bufs=2) as sCpool,    # f32 combine stages
        tc.tile_pool(name="psum", bufs=8, space="PSUM") as pspool,
    ):
        # ---- persistent loads ----
        # DMA priority: phase 1 is gated on ztT + at0, so those go first on
        # separate rings; weights/h/idm are not needed until phase 2.
        ztT = nmpool.tile([CHUNK, NCH * BC], F8, tag="nm", name="ztT")
        nc.scalar.dma_start(out=ztT[:, :], in_=zt_d[:, :])
        # resident adjacency^T (x4096, fp8), one tile per output band;
        # each band split across both HWDGE rings so it lands in ~4us
        at_sb = []
        for g in range(NBAND):
            t = apool.tile([CHUNK, NCH * BW[g]], F8, tag=f"at{g}",
                           name=f"at{g}")
            half = (NCH // 2) * BW[g]
            nc.sync.dma_start(out=t[:, 0:half],
                              in_=at_d[:, AOFF[g]:AOFF[g] + half])
            nc.scalar.dma_start(
                out=t[:, half:NCH * BW[g]],
                in_=at_d[:, AOFF[g] + half:AOFF[g] + NCH * BW[g]])
            at_sb.append(t[:, :].rearrange("p (j m) -> p j m", j=NCH))

        idm = cpool.tile([CHUNK, CHUNK], F16, tag="idm")
        nc.sync.dma_start(out=idm[:], in_=id16_d[:])
        idm8 = cpool.tile([CHUNK, CHUNK], F8, tag="idm8")
        nc.sync.dma_start(out=idm8[:], in_=id8_d[:])
        wf_sb = [cpool.tile([C, D_H], F16, tag=f"wf{k}", name=f"wf{k}")
                 for k in range(3)]
        wu_sb = [cpool.tile([C, D_H], F16, tag=f"wu{k}", name=f"wu{k}")
                 for k in range(3)]
        wcx_sb = [cpool.tile([D_IN, D_H], F16, tag=f"wcx{k}", name=f"wcx{k}")
                  for k in range(3)]
        wcrh_sb = [cpool.tile([D_H, D_H], F16, tag=f"wcrh{k}", name=f"wcrh{k}")
                   for k in range(3)]
        for k in range(3):
            nc.scalar.dma_start(out=wf_sb[k][:], in_=wf_d[k])
            nc.scalar.dma_start(out=wu_sb[k][:], in_=wu_d[k])
            nc.scalar.dma_start(out=wcx_sb[k][:], in_=wcx_d[k])
            nc.scalar.dma_start(out=wcrh_sb[k][:], in_=wcrh_d[k])
        bf_sb = cpool.tile([BH, 1], F32, tag="bf")
        nc.sync.dma_start(out=bf_sb[:], in_=bf_d[:])
        bu_sb = cpool.tile([BH, 1], F32, tag="bu")
        nc.sync.dma_start(out=bu_sb[:], in_=bu_d[:])
        bc_sb = cpool.tile([BH, 1], F32, tag="bc")
        nc.sync.dma_start(out=bc_sb[:], in_=bc_d[:])

        h_st = ppool.tile([BH, NN], F16, tag="h_st")
        nc.scalar.dma_start(out=h_st[:], in_=h_d[:])

        u_st = ppool.tile([BH, NN], F16, tag="u_st")
        rh_st = ppool.tile([BH, NP], F16, tag="rh_st")
        nc.vector.memset(rh_st[:, NN:NP], 0.0)
        c_part = ppool.tile([BH, NN], F16, tag="c_part")
        rhT = ppool.tile([CHUNK, NCH * BH], F8, tag="rhT")
        zc1_bm = ppool.tile([BH, NP], F8, tag="zc1_bm")
        nc.vector.memset(zc1_bm[:, NN:NP], 0.0)
        zc1T = ppool.tile([CHUNK, NCH * BH], F8, tag="zc1T")

        zt3 = ztT[:, :].rearrange("p (j f) -> p j f", j=NCH)
        rhT3 = rhT[:, :].rearrange("p (j f) -> p j f", j=NCH)
        zc1T3 = zc1T[:, :].rearrange("p (j f) -> p j f", j=NCH)

        def sa_band(g, src3, dst3, evac_scale):
            """band g of dst = A @ src, node-major -> node-major."""
            ml = _mlist(g)
            pss = [pspool.tile([CHUNK, BC], F32, tag="ps", name=f"psd{mi}")
                   for mi in range(len(ml))]
            for jj in range(NPR):
                for mi, (mo, mw) in enumerate(ml):
                    nc.tensor.matmul(
                        pss[mi][0:mw, :],
                        lhsT=at_sb[g][:, 2 * jj:2 * jj + 2, mo:mo + mw],
                        rhs=src3[:, 2 * jj:2 * jj + 2, :],
                        start=(jj == 0), stop=(jj == NPR - 1), perf_mode=DR)
            for mi, (mo, mw) in enumerate(ml):
                nc.scalar.activation(dst3[0:mw, g * 4 + mi, :],
                                     pss[mi][0:mw, :], COPY,
                                     scale=evac_scale)

        # ---- phase 1: z1 = A z ----
        z1T = nmpool.tile([CHUNK, NCH * BC], F8, tag="nm", name="z1T")
        z13 = z1T[:, :].rearrange("p (j f) -> p j f", j=NCH)
        nc.vector.memset(z13[:, NCH - 1, :], 0.0)
        for g in range(NBAND):
            sa_band(g, zt3, z13, S_Z1E)

        # ---- phase 2: z2 = A z1, fused with gate convs, rh, rhT ----
        z2T = nmpool.tile([CHUNK, NCH * BC], F8, tag="nm", name="z2T")
        z23 = z2T[:, :].rearrange("p (j f) -> p j f", j=NCH)
        nc.vector.memset(z23[:, NCH - 1, :], 0.0)

        def post2(g):
            ml = _mlist(g)
            m0, w = BOFF[g], BW[g]
            for b in range(B_LOC):
                rows = slice(b * D_H, (b + 1) * D_H)
                # stage conv features (fp16, base partition 0)
                z0s = sBpool.tile([CHUNK, 512], F16, tag="z0s", name="z0s")
                nc.sync.dma_start_transpose(
                    out=z0s[:, 0:w], in_=zn_d[m0:m0 + w, b, :])
                z1s = sApool.tile([C, 512], F16, tag="z1s", name="z1s")
                z2s = sApool.tile([C, 512], F16, tag="z2s", name="z2s")
                for src3, dst in ((z13, z1s), (z23, z2s)):
                    for mi, (mo, mw) in enumerate(ml):
                        # fp8 PE transpose writes PSUM at element step 2
                        pt = pspool.tile([C, 2 * CHUNK], F8, tag="ps",
                                         name="pt")
                        nc.tensor.transpose(
                            pt[:, 0:2 * CHUNK:2],
                            src3[:, g * 4 + mi, b * C:(b + 1) * C],
                            idm8[:, :])
                        nc.vector.tensor_copy(out=dst[:, mo:mo + mw],
                                              in_=pt[:, 0:2 * mw:2])
                feats = (z0s[0:C, 0:w], z1s[:, 0:w], z2s[:, 0:w])
                feats_x = (z0s[0:D_IN, 0:w], z1s[0:D_IN, 0:w],
                           z2s[0:D_IN, 0:w])
                psf = pspool.tile([BH, 512], F32, tag="ps", name="psf") \
                    if b == 0 else psf
                psu = pspool.tile([BH, 512], F32, tag="ps", name="psu") \
                    if b == 0 else psu
                psx = pspool.tile([BH, 512], F32, tag="ps", name="psx") \
                    if b == 0 else psx
                for k in range(3):
                    nc.tensor.matmul(psf[rows, 0:w], lhsT=wf_sb[k],
                                     rhs=feats[k], start=(k == 0),
                                     stop=(k == 2))
                for k in range(3):
                    nc.tensor.matmul(psu[rows, 0:w], lhsT=wu_sb[k],
                                     rhs=feats[k], start=(k == 0),
                                     stop=(k == 2))
                for k in range(3):
                    nc.tensor.matmul(psx[rows, 0:w], lhsT=wcx_sb[k],
                                     rhs=feats_x[k], start=(k == 0),
                                     stop=(k == 2))
            # gate nonlinearities + rh, full 128 partitions
            rst = sBpool.tile([BH, 512], F16, tag="rst", name="rst")
            nc.scalar.activation(rst[:, 0:w], psf[:, 0:w], SIG, bias=bf_sb[:, :])
            nc.scalar.activation(u_st[:, m0:m0 + w], psu[:, 0:w], SIG,
                                 bias=bu_sb[:, :])
            nc.vector.tensor_mul(out=rh_st[:, m0:m0 + w], in0=rst[:, 0:w],
                                 in1=h_st[:, m0:m0 + w])
            nc.vector.tensor_copy(out=c_part[:, m0:m0 + w], in_=psx[:, 0:w])
            # rhT for the candidate diffusion (node-major fp8)
            for b in range(B_LOC):
                rows = slice(b * D_H, (b + 1) * D_H)
                for mi, (mo, mw) in enumerate(ml):
                    ch = g * 4 + mi
                    ptr = pspool.tile([CHUNK, D_H], F16, tag="ps", name="ptr")
                    nc.tensor.transpose(
                        ptr[:, :],
                        rh_st[rows, ch * CHUNK:(ch + 1) * CHUNK],
                        idm[rows, rows])
                    nc.vector.tensor_copy(
                        out=rhT3[:, ch, b * D_H:(b + 1) * D_H], in_=ptr[:, :])

        # phase 2 driver: band g's dependent work staggered one band behind
        # the sa matmuls so the ACT/DVE round trips hide under PE work
        for g in range(NBAND):
            sa_band(g, z13, z23, S_Z2E)
            if g > 0:
                post2(g - 1)
        post2(NBAND - 1)

        # ---- phase 3: zc1 = A rh (activations stationary, adj moving) ----
        def mm_sz(g, lhsT3, name):
            psc = pspool.tile([BH, 512], F32, tag="ps", name=name)
            w = BW[g]
            for jj in range(NPR):
                nc.tensor.matmul(
                    psc[:, 0:w],
                    lhsT=lhsT3[:, 2 * jj:2 * jj + 2, :],
                    rhs=at_sb[g][:, 2 * jj:2 * jj + 2, 0:w],
                    start=(jj == 0), stop=(jj == NPR - 1), perf_mode=DR)
            return psc

        def post3(g):
            for b in range(B_LOC):
                rows = slice(b * D_H, (b + 1) * D_H)
                for mi, (mo, mw) in enumerate(_mlist(g)):
                    ch = g * 4 + mi
                    ptc = pspool.tile([CHUNK, 2 * D_H], F8, tag="ps",
                                      name="ptc")
                    nc.tensor.transpose(
                        ptc[:, 0:2 * D_H:2],
                        zc1_bm[rows, ch * CHUNK:(ch + 1) * CHUNK],
                        idm8[rows, rows])
                    nc.vector.tensor_copy(
                        out=zc1T3[:, ch, b * D_H:(b + 1) * D_H],
                        in_=ptc[:, 0:2 * D_H:2])

        for g in range(NBAND):
            m0, w = BOFF[g], BW[g]
            psc = mm_sz(g, rhT3, "psc")
            nc.scalar.activation(zc1_bm[:, m0:m0 + w], psc[:, 0:w], COPY,
                                 scale=S_C1E)
            if g > 0:
                post3(g - 1)
        post3(NBAND - 1)

        # ---- phase 4: zc2 = A zc1, fused candidate conv + combine ----
        def cons4(g, psc2):
            m0, w = BOFF[g], BW[g]
            zc2s = sBpool.tile([BH, 512], F16, tag="zc2s", name="zc2s")
            nc.scalar.activation(zc2s[:, 0:w], psc2[:, 0:w], COPY,
                                 scale=S_C2E)
            zc1s = sBpool.tile([BH, 512], F16, tag="zc1s", name="zc1s")
            nc.vector.tensor_copy(out=zc1s[:, 0:w], in_=zc1_bm[:, m0:m0 + w])
            # batch-1 features need base partition 0: SBUF->SBUF DMA restage
            b1rh = sApool.tile([D_H, 512], F16, tag="b1rh", name="b1rh")
            nc.scalar.dma_start(out=b1rh[:, 0:w], in_=rh_st[D_H:BH, m0:m0 + w])
            b1c1 = sApool.tile([D_H, 512], F16, tag="b1c1", name="b1c1")
            nc.scalar.dma_start(out=b1c1[:, 0:w], in_=zc1s[D_H:BH, 0:w])
            b1c2 = sApool.tile([D_H, 512], F16, tag="b1c2", name="b1c2")
            nc.scalar.dma_start(out=b1c2[:, 0:w], in_=zc2s[D_H:BH, 0:w])
            psc3 = pspool.tile([BH, 512], F32, tag="ps", name="psc3")
            for b in range(B_LOC):
                rows = slice(b * D_H, (b + 1) * D_H)
                terms = ((rh_st[0:D_H, m0:m0 + w], zc1s[0:D_H, 0:w],
                          zc2s[0:D_H, 0:w]) if b == 0 else
                         (b1rh[:, 0:w], b1c1[:, 0:w], b1c2[:, 0:w]))
                for k in range(3):
                    nc.tensor.matmul(psc3[rows, 0:w], lhsT=wcrh_sb[k],
                                     rhs=terms[k], start=(k == 0),
                                     stop=(k == 2))
            tt = sCpool.tile([BH, 512], F16, tag="tt", name="tt")
            nc.vector.tensor_add(out=tt[:, 0:w], in0=psc3[:, 0:w],
                                 in1=c_part[:, m0:m0 + w])
            cst = sCpool.tile([BH, 512], F32, tag="cst", name="cst")
            nc.scalar.activation(cst[:, 0:w], tt[:, 0:w], TANH, bias=bc_sb[:, :])
            # combine on the otherwise-idle Pool engine; tt reused for h-c
            nc.gpsimd.tensor_sub(out=tt[:, 0:w], in0=h_st[:, m0:m0 + w],
                                 in1=cst[:, 0:w])
            nc.gpsimd.tensor_mul(out=tt[:, 0:w], in0=u_st[:, m0:m0 + w],
                                 in1=tt[:, 0:w])
            nc.gpsimd.tensor_add(out=cst[:, 0:w], in0=cst[:, 0:w],
                                 in1=tt[:, 0:w])
            for b in range(B_LOC):
                nc.sync.dma_start(
                    out=out_d[b][:, m0:m0 + w],
                    in_=cst[b * D_H:(b + 1) * D_H, 0:w])

        psc2_prev = None
        for g in range(NBAND):
            psc2 = mm_sz(g, zc1T3, "psc2")
            if g > 0:
                cons4(g - 1, psc2_prev)
            psc2_prev = psc2
        cons4(NBAND - 1, psc2_prev)


# ---- host-side driver ----
_CACHED_NC = None
TRACE = False           # set True (e.g. from test.py) to capture an NTFF profile
TRACE_DIR = None
LAST_RESULTS = None     # BassKernelResults of the most recent kernel() call


def _host_prep(x, h, adj, Wf, bf, Wu, bu, Wc, bc):
    """Shard + cast + layout inputs for the 8 cores. Returns list of in_maps."""
    atp = np.zeros((NP, NN), dtype=np.float32)
    atp[:NN] = adj.T * 4096.0
    at8 = atp.astype(E4NP)                       # [4096, 4000]
    blocks = at8.reshape(NCH, CHUNK, NN)
    cols = [np.ascontiguousarray(
        blocks[:, :, BOFF[g]:BOFF[g] + BW[g]].transpose(1, 0, 2)
    ).reshape(CHUNK, NCH * BW[g]) for g in range(NBAND)]
    at_h = np.ascontiguousarray(np.concatenate(cols, axis=1))

    id16 = np.eye(CHUNK, dtype=np.float16)
    id8 = np.eye(CHUNK, dtype=E4NP)

    wsc = {"wf": (1.0, 1 / 32., 1 / 512.), "wu": (1.0, 1 / 32., 1 / 512.),
           "wcx": (1.0, 1 / 32., 1 / 512.), "wcrh": (1.0, 1 / 64., 1 / 512.)}

    def wsplit(W, key, lo, hi):
        return np.ascontiguousarray(np.stack(
            [(W[:, k * C + lo:k * C + hi].T * wsc[key][k]).astype(np.float16)
             for k in range(3)]))

    def bstack(v):
        return np.concatenate([v] * B_LOC).reshape(BH, 1).astype(np.float32)

    shared = {
        "at": at_h, "id16": id16, "id8": id8,
        "wf": wsplit(Wf, "wf", 0, C), "wu": wsplit(Wu, "wu", 0, C),
        "wcx": wsplit(Wc, "wcx", 0, D_IN), "wcrh": wsplit(Wc, "wcrh", D_IN, C),
        "bf": bstack(bf), "bu": bstack(bu), "bcb": bstack(bc),
    }
    in_maps = []
    for core in range(NCORES):
        bs = slice(core * B_LOC, (core + 1) * B_LOC)
        z = np.concatenate([x[bs], h[bs]], axis=1)       # [B_LOC, C, NN]
        znm = z.transpose(2, 0, 1)                       # [NN, B_LOC, C]
        ztp = np.zeros((NP, BC), dtype=np.float32)
        ztp[:NN] = znm.reshape(NN, BC)
        zt8 = np.ascontiguousarray(
            ztp.astype(E4NP).reshape(NCH, CHUNK, BC).transpose(1, 0, 2)
        ).reshape(CHUNK, NCH * BC)
        znp = np.zeros((NP, B_LOC, CHUNK), dtype=np.float16)
        znp[:NN, :, :C] = znm
        h_p = np.ascontiguousarray(
            h[bs].astype(np.float16).reshape(BH, NN))
        in_maps.append(dict(shared, zt=zt8, zn=znp, h=h_p))
    return in_maps


def kernel(**inputs):
    global _CACHED_NC, LAST_RESULTS
    inputs = {k: np.asarray(v) for k, v in inputs.items()}
    if _CACHED_NC is None:
        _CACHED_NC = build_program()
    in_maps = _host_prep(**inputs)
    kw = {}
    if TRACE:
        kw = dict(trace=True, tmpdir=TRACE_DIR)
    res = run_bass_kernel_spmd(_CACHED_NC, in_maps,
                               core_ids=list(range(NCORES)), **kw)
    LAST_RESULTS = res
    outs = [res.results[i]["out"] for i in range(NCORES)]
    return np.concatenate(outs, axis=0).astype(np.float32)


if __name__ == "__main__":
    rng = np.random.default_rng(0)
    ins = {
        "x": rng.standard_normal((B, D_IN, NN), dtype=np.float32),
        "h": rng.standard_normal((B, D_H, NN), dtype=np.float32),
        "adj": rng.random((NN, NN), dtype=np.float32) / NN,
        "Wf": rng.standard_normal((D_H, 3 * C), dtype=np.float32) * 0.05,
        "Wu": rng.standard_normal((D_H, 3 * C), dtype=np.float32) * 0.05,
        "Wc": rng.standard_normal((D_H, 3 * C), dtype=np.float32) * 0.05,
        "bf": rng.standard_normal(D_H).astype(np.float32) * 0.05,
        "bu": rng.standard_normal(D_H).astype(np.float32) * 0.05,
        "bc": rng.standard_normal(D_H).astype(np.float32) * 0.05,
    }
    out = kernel(**ins)
    print(out.shape, out.dtype)


# revision 21
# speedup vs baseline: 1.0930x; 1.0930x over previous
"""GCGRU cell (order-2 graph diffusion GRU) Trainium2 Bass kernel, v2.

Strategy: data-parallel over batch (B=16 -> 2 batches per core x 8 cores).
The dominant cost in v1 was streaming the 32MB fp16 adjacency from HBM four
times per core (DMA 99% busy). v2 keeps the whole adjacency RESIDENT in SBUF
as fp8 (x4096 pre-scale keeps the row-normalized values out of e4m3's
denormal range), loaded once (~16MB), and runs all four diffusion passes as
fp8 DoubleRow matmuls (2 packed contraction rows/cycle). Diffused features
are small contributors to the output (the graph averages 4000 nodes), so fp8
error lands ~1e-4 relative; order-k features carry power-of-2 scales folded
into the PSUM-evacuation copies and undone by host-side weight pre-scaling.

Layouts per core: activations node-major fp8 [128p x (chunk, col)] for
diffusion; gate/candidate convs run fp16 from per-band staging tiles
(PE transposes for diffused features, XBAR DMA-transpose from DRAM for the
raw [x;h] features). Gate/candidate nonlinearities on ACT, elementwise on
DVE, combine fused into the last diffusion's band loop.
"""

import numpy as np
import ml_dtypes

import concourse.bass as bass
from concourse import bacc
import concourse.mybir as mybir
import concourse.tile as tile
from concourse.bass_utils import run_bass_kernel_spmd

# problem constants
B, D_IN, D_H, NN = 16, 32, 64, 4000
NCORES = 8
B_LOC = B // NCORES          # batches per core
C = D_IN + D_H               # 96 channels into each gate conv
BC = B_LOC * C               # node-major column count (b-major: [b0 c96 | b1 c96])
BH = B_LOC * D_H             # stacked batch-hidden rows (128)
NP = 4096                    # contraction node dim padded to 32 chunks
CHUNK = 128
NCH = NP // CHUNK            # 32 contraction chunks
NPR = NCH // 2               # 16 DoubleRow chunk pairs
NBAND = 8                    # output-node bands: 7x512 + 416 (= 4000, no pad)
BW = [512] * 7 + [416]
BOFF = [512 * g for g in range(NBAND)]
AOFF = [NCH * 512 * g for g in range(NBAND)]   # at_d col offset per band

F8 = mybir.dt.float8e4
F16 = mybir.dt.float16
F32 = mybir.dt.float32
DR = mybir.MatmulPerfMode.DoubleRow
E4NP = ml_dtypes.float8_e4m3

# fp8 scale chain: adjacency carries x4096 (2^12).
#   z1T carries x32   -> evac scale 32/4096
#   z2T carries x512  -> evac scale 512/(4096*32)
#   zc1 carries x64   -> evac scale 64/4096
#   zc2 stage x512    -> evac scale 512/(4096*64)
S_Z1E, S_Z2E = 2.0 ** -7, 2.0 ** -8
S_C1E, S_C2E = 2.0 ** -6, 2.0 ** -9
# matching host-side weight descales: gate W1 /32, W2 /512; cand x-part
# W1 /32, W2 /512; cand rh-part W1 /64, W2 /512.


def _mlist(g):
    """(offset, width) of the 128-wide m-chunks inside band g."""
    w = BW[g]
    out = []
    mo = 0
    while mo < w:
        out.append((mo, min(CHUNK, w - mo)))
        mo += CHUNK
    return out


def build_program():
    nc = bacc.Bacc("TRN2", target_bir_lowering=False, debug=False)

    at_d = nc.dram_tensor("at", [CHUNK, NCH * NN], F8, kind="ExternalInput").ap()
    zt_d = nc.dram_tensor("zt", [CHUNK, NCH * BC], F8, kind="ExternalInput").ap()
    # node-major [x;h] fp16, padded to 128 cols/batch for XBAR dma transpose
    zn_d = nc.dram_tensor("zn", [NP, B_LOC, CHUNK], F16, kind="ExternalInput").ap()
    h_d = nc.dram_tensor("h", [BH, NN], F16, kind="ExternalInput").ap()
    wf_d = nc.dram_tensor("wf", [3, C, D_H], F16, kind="ExternalInput").ap()
    wu_d = nc.dram_tensor("wu", [3, C, D_H], F16, kind="ExternalInput").ap()
    wcx_d = nc.dram_tensor("wcx", [3, D_IN, D_H], F16, kind="ExternalInput").ap()
    wcrh_d = nc.dram_tensor("wcrh", [3, D_H, D_H], F16, kind="ExternalInput").ap()
    bf_d = nc.dram_tensor("bf", [BH, 1], F32, kind="ExternalInput").ap()
    bu_d = nc.dram_tensor("bu", [BH, 1], F32, kind="ExternalInput").ap()
    bc_d = nc.dram_tensor("bcb", [BH, 1], F32, kind="ExternalInput").ap()
    id16_d = nc.dram_tensor("id16", [CHUNK, CHUNK], F16, kind="ExternalInput").ap()
    id8_d = nc.dram_tensor("id8", [CHUNK, CHUNK], F8, kind="ExternalInput").ap()
    out_d = nc.dram_tensor("out", [B_LOC, D_H, NN], F32, kind="ExternalOutput").ap()

    with tile.TileContext(nc) as tc:
        _body(tc, locals())
    nc.compile()
    return nc


def _body(tc, aps):
    nc = tc.nc
    at_d, zt_d, zn_d, h_d = aps["at_d"], aps["zt_d"], aps["zn_d"], aps["h_d"]
    wf_d, wu_d, wcx_d, wcrh_d = (
        aps["wf_d"], aps["wu_d"], aps["wcx_d"], aps["wcrh_d"])
    bf_d, bu_d, bc_d = aps["bf_d"], aps["bu_d"], aps["bc_d"]
    id16_d, id8_d, out_d = aps["id16_d"], aps["id8_d"], aps["out_d"]

    SIG = mybir.ActivationFunctionType.Sigmoid
    TANH = mybir.ActivationFunctionType.Tanh
    COPY = mybir.ActivationFunctionType.Copy

    with (
        tc.tile_pool(name="const", bufs=1) as cpool,
        tc.tile_pool(name="amat", bufs=1) as apool,       # resident adjacency
        tc.tile_pool(name="nm8", bufs=2) as nmpool,       # rotating node-major fp8
        tc.tile_pool(name="perst", bufs=1) as ppool,
        tc.tile_pool(name="stageA", bufs=2) as sApool,    # conv feature stages
        tc.tile_pool(name="stageB", bufs=2) as sBpool,    # wide f16 stages
        tc.tile_pool(name="stageC", bufs=2) as sCpool,    # f32 combine stages
        tc.tile_pool(name="psum", bufs=8, space="PSUM") as pspool,
    ):
        # ---- persistent loads ----
        # DMA priority: phase 1 is gated on ztT + at0, so those go first on
        # separate rings; weights/h/idm are not needed until phase 2.
        # small loads first — they must not queue behind the ring-throttled
        # adjacency triggers
        idm = cpool.tile([CHUNK, CHUNK], F16, tag="idm")
        nc.sync.dma_start(out=idm[:], in_=id16_d[:])
        idm8 = cpool.tile([CHUNK, CHUNK], F8, tag="idm8")
        nc.sync.dma_start(out=idm8[:], in_=id8_d[:])
        bf_sb = cpool.tile([BH, 1], F32, tag="bf")
        nc.sync.dma_start(out=bf_sb[:], in_=bf_d[:])
        bu_sb = cpool.tile([BH, 1], F32, tag="bu")
        nc.sync.dma_start(out=bu_sb[:], in_=bu_d[:])
        bc_sb = cpool.tile([BH, 1], F32, tag="bc")
        nc.sync.dma_start(out=bc_sb[:], in_=bc_d[:])
        wf_sb = [cpool.tile([C, D_H], F16, tag=f"wf{k}", name=f"wf{k}")
                 for k in range(3)]
        wu_sb = [cpool.tile([C, D_H], F16, tag=f"wu{k}", name=f"wu{k}")
                 for k in range(3)]
        wcx_sb = [cpool.tile([D_IN, D_H], F16, tag=f"wcx{k}", name=f"wcx{k}")
                  for k in range(3)]
        wcrh_sb = [cpool.tile([D_H, D_H], F16, tag=f"wcrh{k}", name=f"wcrh{k}")
                   for k in range(3)]
        for k in range(3):
            nc.scalar.dma_start(out=wf_sb[k][:], in_=wf_d[k])
            nc.scalar.dma_start(out=wu_sb[k][:], in_=wu_d[k])
            nc.scalar.dma_start(out=wcx_sb[k][:], in_=wcx_d[k])
            nc.scalar.dma_start(out=wcrh_sb[k][:], in_=wcrh_d[k])
        h_st = ppool.tile([BH, NN], F16, tag="h_st")
        nc.scalar.dma_start(out=h_st[:], in_=h_d[:])

        ztT = nmpool.tile([CHUNK, NCH * BC], F8, tag="nm", name="ztT")
        nc.scalar.dma_start(out=ztT[:, :], in_=zt_d[:, :])
        # resident adjacency^T (x4096, fp8), one tile per output band;
        # each band split across both HWDGE rings so it lands in ~4us
        at_sb = []
        for g in range(NBAND):
            t = apool.tile([CHUNK, NCH * BW[g]], F8, tag=f"at{g}",
                           name=f"at{g}")
            half = (NCH // 2) * BW[g]
            nc.sync.dma_start(out=t[:, 0:half],
                              in_=at_d[:, AOFF[g]:AOFF[g] + half])
            nc.scalar.dma_start(
                out=t[:, half:NCH * BW[g]],
                in_=at_d[:, AOFF[g] + half:AOFF[g] + NCH * BW[g]])
            at_sb.append(t[:, :].rearrange("p (j m) -> p j m", j=NCH))

        u_st = ppool.tile([BH, NN], F16, tag="u_st")
        rh_st = ppool.tile([BH, NP], F16, tag="rh_st")
        nc.vector.memset(rh_st[:, NN:NP], 0.0)
        c_part = ppool.tile([BH, NN], F16, tag="c_part")
        rhT = ppool.tile([CHUNK, NCH * BH], F8, tag="rhT")
        zc1_bm = ppool.tile([BH, NP], F8, tag="zc1_bm")
        nc.vector.memset(zc1_bm[:, NN:NP], 0.0)
        zc1T = ppool.tile([CHUNK, NCH * BH], F8, tag="zc1T")

        zt3 = ztT[:, :].rearrange("p (j f) -> p j f", j=NCH)
        rhT3 = rhT[:, :].rearrange("p (j f) -> p j f", j=NCH)
        zc1T3 = zc1T[:, :].rearrange("p (j f) -> p j f", j=NCH)

        def sa_band(g, src3, dst3, evac_scale):
            """band g of dst = A @ src, node-major -> node-major."""
            ml = _mlist(g)
            pss = [pspool.tile([CHUNK, BC], F32, tag="ps", name=f"psd{mi}")
                   for mi in range(len(ml))]
            for jj in range(NPR):
                for mi, (mo, mw) in enumerate(ml):
                    nc.tensor.matmul(
                        pss[mi][0:mw, :],
                        lhsT=at_sb[g][:, 2 * jj:2 * jj + 2, mo:mo + mw],
                        rhs=src3[:, 2 * jj:2 * jj + 2, :],
                        start=(jj == 0), stop=(jj == NPR - 1), perf_mode=DR)
            for mi, (mo, mw) in enumerate(ml):
                # evac on DVE: the ACT queue holds the ring-throttled
                # adjacency DMA triggers early on and must not gate PSUM reuse
                nc.vector.tensor_scalar_mul(
                    out=dst3[0:mw, g * 4 + mi, :], in0=pss[mi][0:mw, :],
                    scalar1=evac_scale)

        # ---- phase 1: z1 = A z ----
        z1T = nmpool.tile([CHUNK, NCH * BC], F8, tag="nm", name="z1T")
        z13 = z1T[:, :].rearrange("p (j f) -> p j f", j=NCH)
        nc.vector.memset(z13[:, NCH - 1, :], 0.0)
        for g in range(NBAND):
            sa_band(g, zt3, z13, S_Z1E)

        # ---- phase 2: z2 = A z1, fused with gate convs, rh, rhT ----
        z2T = nmpool.tile([CHUNK, NCH * BC], F8, tag="nm", name="z2T")
        z23 = z2T[:, :].rearrange("p (j f) -> p j f", j=NCH)
        nc.vector.memset(z23[:, NCH - 1, :], 0.0)

        def post2(g):
            ml = _mlist(g)
            m0, w = BOFF[g], BW[g]
            for b in range(B_LOC):
                rows = slice(b * D_H, (b + 1) * D_H)
                # stage conv features (fp16, base partition 0)
                z0s = sBpool.tile([CHUNK, 512], F16, tag="z0s", name="z0s")
                nc.sync.dma_start_transpose(
                    out=z0s[:, 0:w], in_=zn_d[m0:m0 + w, b, :])
                z1s = sApool.tile([C, 512], F16, tag="z1s", name="z1s")
                z2s = sApool.tile([C, 512], F16, tag="z2s", name="z2s")
                for src3, dst in ((z13, z1s), (z23, z2s)):
                    for mi, (mo, mw) in enumerate(ml):
                        # fp8 PE transpose writes PSUM at element step 2
                        pt = pspool.tile([C, 2 * CHUNK], F8, tag="ps",
                                         name="pt")
                        nc.tensor.transpose(
                            pt[:, 0:2 * CHUNK:2],
                            src3[:, g * 4 + mi, b * C:(b + 1) * C],
                            idm8[:, :])
                        nc.vector.tensor_copy(out=dst[:, mo:mo + mw],
                                              in_=pt[:, 0:2 * mw:2])
                feats = (z0s[0:C, 0:w], z1s[:, 0:w], z2s[:, 0:w])
                feats_x = (z0s[0:D_IN, 0:w], z1s[0:D_IN, 0:w],
                           z2s[0:D_IN, 0:w])
                psf = pspool.tile([BH, 512], F32, tag="ps", name="psf") \
                    if b == 0 else psf
                psu = pspool.tile([BH, 512], F32, tag="ps", name="psu") \
                    if b == 0 else psu
                psx = pspool.tile([BH, 512], F32, tag="ps", name="psx") \
                    if b == 0 else psx
                for k in range(3):
                    nc.tensor.matmul(psf[rows, 0:w], lhsT=wf_sb[k],
                                     rhs=feats[k], start=(k == 0),
                                     stop=(k == 2))
                for k in range(3):
                    nc.tensor.matmul(psu[rows, 0:w], lhsT=wu_sb[k],
                                     rhs=feats[k], start=(k == 0),
                                     stop=(k == 2))
                for k in range(3):
                    nc.tensor.matmul(psx[rows, 0:w], lhsT=wcx_sb[k],
                                     rhs=feats_x[k], start=(k == 0),
                                     stop=(k == 2))
            # gate nonlinearities + rh, full 128 partitions
            rst = sBpool.tile([BH, 512], F16, tag="rst", name="rst")
            nc.scalar.activation(rst[:, 0:w], psf[:, 0:w], SIG, bias=bf_sb[:, :])
            nc.scalar.activation(u_st[:, m0:m0 + w], psu[:, 0:w], SIG,
                                 bias=bu_sb[:, :])
            nc.vector.tensor_mul(out=rh_st[:, m0:m0 + w], in0=rst[:, 0:w],
                                 in1=h_st[:, m0:m0 + w])
            nc.vector.tensor_copy(out=c_part[:, m0:m0 + w], in_=psx[:, 0:w])
            # rhT for the candidate diffusion (node-major fp8)
            for b in range(B_LOC):
                rows = slice(b * D_H, (b + 1) * D_H)
                for mi, (mo, mw) in enumerate(ml):
                    ch = g * 4 + mi
                    ptr = pspool.tile([CHUNK, D_H], F16, tag="ps", name="ptr")
                    nc.tensor.transpose(
                        ptr[:, :],
                        rh_st[rows, ch * CHUNK:(ch + 1) * CHUNK],
                        idm[rows, rows])
                    nc.vector.tensor_copy(
                        out=rhT3[:, ch, b * D_H:(b + 1) * D_H], in_=ptr[:, :])

        # phase 2 driver: band g's dependent work staggered one band behind
        # the sa matmuls so the ACT/DVE round trips hide under PE work
        for g in range(NBAND):
            sa_band(g, z13, z23, S_Z2E)
            if g > 0:
                post2(g - 1)
        post2(NBAND - 1)

        # ---- phase 3: zc1 = A rh (activations stationary, adj moving) ----
        def mm_sz(g, lhsT3, name):
            psc = pspool.tile([BH, 512], F32, tag="ps", name=name)
            w = BW[g]
            for jj in range(NPR):
                nc.tensor.matmul(
                    psc[:, 0:w],
                    lhsT=lhsT3[:, 2 * jj:2 * jj + 2, :],
                    rhs=at_sb[g][:, 2 * jj:2 * jj + 2, 0:w],
                    start=(jj == 0), stop=(jj == NPR - 1), perf_mode=DR)
            return psc

        def post3(g):
            for b in range(B_LOC):
                rows = slice(b * D_H, (b + 1) * D_H)
                for mi, (mo, mw) in enumerate(_mlist(g)):
                    ch = g * 4 + mi
                    ptc = pspool.tile([CHUNK, 2 * D_H], F8, tag="ps",
                                      name="ptc")
                    nc.tensor.transpose(
                        ptc[:, 0:2 * D_H:2],
                        zc1_bm[rows, ch * CHUNK:(ch + 1) * CHUNK],
                        idm8[rows, rows])
                    nc.vector.tensor_copy(
                        out=zc1T3[:, ch, b * D_H:(b + 1) * D_H],
                        in_=ptc[:, 0:2 * D_H:2])

        for g in range(NBAND):
            m0, w = BOFF[g], BW[g]
            psc = mm_sz(g, rhT3, "psc")
            nc.scalar.activation(zc1_bm[:, m0:m0 + w], psc[:, 0:w], COPY,
                                 scale=S_C1E)
            if g > 0:
                post3(g - 1)
        post3(NBAND - 1)

        # ---- phase 4: zc2 = A zc1, fused candidate conv + combine ----
        def cons4(g, psc2):
            m0, w = BOFF[g], BW[g]
            zc2s = sBpool.tile([BH, 512], F16, tag="zc2s", name="zc2s")
            nc.scalar.activation(zc2s[:, 0:w], psc2[:, 0:w], COPY,
                                 scale=S_C2E)
            zc1s = sBpool.tile([BH, 512], F16, tag="zc1s", name="zc1s")
            nc.vector.tensor_copy(out=zc1s[:, 0:w], in_=zc1_bm[:, m0:m0 + w])
            # batch-1 features need base partition 0: SBUF->SBUF DMA restage
            b1rh = sApool.tile([D_H, 512], F16, tag="b1rh", name="b1rh")
            nc.scalar.dma_start(out=b1rh[:, 0:w], in_=rh_st[D_H:BH, m0:m0 + w])
            b1c1 = sApool.tile([D_H, 512], F16, tag="b1c1", name="b1c1")
            nc.scalar.dma_start(out=b1c1[:, 0:w], in_=zc1s[D_H:BH, 0:w])
            b1c2 = sApool.tile([D_H, 512], F16, tag="b1c2", name="b1c2")
            nc.scalar.dma_start(out=b1c2[:, 0:w], in_=zc2s[D_H:BH, 0:w])
            psc3 = pspool.tile([BH, 512], F32, tag="ps", name="psc3")
            for b in range(B_LOC):
                rows = slice(b * D_H, (b + 1) * D_H)
                terms = ((rh_st[0:D_H, m0:m0 + w], zc1s[0:D_H, 0:w],
                          zc2s[0:D_H, 0:w]) if b == 0 else
                         (b1rh[:, 0:w], b1c1[:, 0:w], b1c2[:, 0:w]))
                for k in range(3):
                    nc.tensor.matmul(psc3[rows, 0:w], lhsT=wcrh_sb[k],
                                     rhs=terms[k], start=(k == 0),
                                     stop=(k == 2))
            tt = sCpool.tile([BH, 512], F16, tag="tt", name="tt")
            nc.vector.tensor_add(out=tt[:, 0:w], in0=psc3[:, 0:w],
                                 in1=c_part[:, m0:m0 + w])
            cst = sCpool.tile([BH, 512], F32, tag="cst", name="cst")
            nc.scalar.activation(cst[:, 0:w], tt[:, 0:w], TANH, bias=bc_sb[:, :])
            # combine on the otherwise-idle Pool engine; tt reused for h-c
            nc.gpsimd.tensor_sub(out=tt[:, 0:w], in0=h_st[:, m0:m0 + w],
                                 in1=cst[:, 0:w])
            nc.gpsimd.tensor_mul(out=tt[:, 0:w], in0=u_st[:, m0:m0 + w],
                                 in1=tt[:, 0:w])
            nc.gpsimd.tensor_add(out=cst[:, 0:w], in0=cst[:, 0:w],
                                 in1=tt[:, 0:w])
            for b in range(B_LOC):
                nc.sync.dma_start(
                    out=out_d[b][:, m0:m0 + w],
                    in_=cst[b * D_H:(b + 1) * D_H, 0:w])

        psc2_prev = None
        for g in range(NBAND):
            psc2 = mm_sz(g, zc1T3, "psc2")
            if g > 0:
                cons4(g - 1, psc2_prev)
            psc2_prev = psc2
        cons4(NBAND - 1, psc2_prev)


# ---- host-side driver ----
_CACHED_NC = None
TRACE = False           # set True (e.g. from test.py) to capture an NTFF profile
TRACE_DIR = None
LAST_RESULTS = None     # BassKernelResults of the most recent kernel() call


def _host_prep(x, h, adj, Wf, bf, Wu, bu, Wc, bc):
    """Shard + cast + layout inputs for the 8 cores. Returns list of in_maps."""
    atp = np.zeros((NP, NN), dtype=np.float32)
    atp[:NN] = adj.T * 4096.0
    at8 = atp.astype(E4NP)                       # [4096, 4000]
    blocks = at8.reshape(NCH, CHUNK, NN)
    cols = [np.ascontiguousarray(
        blocks[:, :, BOFF[g]:BOFF[g] + BW[g]].transpose(1, 0, 2)
    ).reshape(CHUNK, NCH * BW[g]) for g in range(NBAND)]
    at_h = np.ascontiguousarray(np.concatenate(cols, axis=1))

    id16 = np.eye(CHUNK, dtype=np.float16)
    id8 = np.eye(CHUNK, dtype=E4NP)

    wsc = {"wf": (1.0, 1 / 32., 1 / 512.), "wu": (1.0, 1 / 32., 1 / 512.),
           "wcx": (1.0, 1 / 32., 1 / 512.), "wcrh": (1.0, 1 / 64., 1 / 512.)}

    def wsplit(W, key, lo, hi):
        return np.ascontiguousarray(np.stack(
            [(W[:, k * C + lo:k * C + hi].T * wsc[key][k]).astype(np.float16)
             for k in range(3)]))

    def bstack(v):
        return np.concatenate([v] * B_LOC).reshape(BH, 1).astype(np.float32)

    shared = {
        "at": at_h, "id16": id16, "id8": id8,
        "wf": wsplit(Wf, "wf", 0, C), "wu": wsplit(Wu, "wu", 0, C),
        "wcx": wsplit(Wc, "wcx", 0, D_IN), "wcrh": wsplit(Wc, "wcrh", D_IN, C),
        "bf": bstack(bf), "bu": bstack(bu), "bcb": bstack(bc),
    }
    in_maps = []
    for core in range(NCORES):
        bs = slice(core * B_LOC, (core + 1) * B_LOC)
        z = np.concatenate([x[bs], h[bs]], axis=1)       # [B_LOC, C, NN]
        znm = z.transpose(2, 0, 1)                       # [NN, B_LOC, C]
        ztp = np.zeros((NP, BC), dtype=np.float32)
        ztp[:NN] = znm.reshape(NN, BC)
        zt8 = np.ascontiguousarray(
            ztp.astype(E4NP).reshape(NCH, CHUNK, BC).transpose(1, 0, 2)
        ).reshape(CHUNK, NCH * BC)
        znp = np.zeros((NP, B_LOC, CHUNK), dtype=np.float16)
        znp[:NN, :, :C] = znm
        h_p = np.ascontiguousarray(
            h[bs].astype(np.float16).reshape(BH, NN))
        in_maps.append(dict(shared, zt=zt8, zn=znp, h=h_p))
    return in_maps


def kernel(**inputs):
    global _CACHED_NC, LAST_RESULTS
    inputs = {k: np.asarray(v) for k, v in inputs.items()}
    if _CACHED_NC is None:
        _CACHED_NC = build_program()
    in_maps = _host_prep(**inputs)
    kw = {}
    if TRACE:
        kw = dict(trace=True, tmpdir=TRACE_DIR)
    res = run_bass_kernel_spmd(_CACHED_NC, in_maps,
                               core_ids=list(range(NCORES)), **kw)
    LAST_RESULTS = res
    outs = [res.results[i]["out"] for i in range(NCORES)]
    return np.concatenate(outs, axis=0).astype(np.float32)


if __name__ == "__main__":
    rng = np.random.default_rng(0)
    ins = {
        "x": rng.standard_normal((B, D_IN, NN), dtype=np.float32),
        "h": rng.standard_normal((B, D_H, NN), dtype=np.float32),
        "adj": rng.random((NN, NN), dtype=np.float32) / NN,
        "Wf": rng.standard_normal((D_H, 3 * C), dtype=np.float32) * 0.05,
        "Wu": rng.standard_normal((D_H, 3 * C), dtype=np.float32) * 0.05,
        "Wc": rng.standard_normal((D_H, 3 * C), dtype=np.float32) * 0.05,
        "bf": rng.standard_normal(D_H).astype(np.float32) * 0.05,
        "bu": rng.standard_normal(D_H).astype(np.float32) * 0.05,
        "bc": rng.standard_normal(D_H).astype(np.float32) * 0.05,
    }
    out = kernel(**ins)
    print(out.shape, out.dtype)


# revision 24
# speedup vs baseline: 1.0986x; 1.0051x over previous
"""GCGRU cell (order-2 graph diffusion GRU) Trainium2 Bass kernel, v2.

Strategy: data-parallel over batch (B=16 -> 2 batches per core x 8 cores).
The dominant cost in v1 was streaming the 32MB fp16 adjacency from HBM four
times per core (DMA 99% busy). v2 keeps the whole adjacency RESIDENT in SBUF
as fp8 (x4096 pre-scale keeps the row-normalized values out of e4m3's
denormal range), loaded once (~16MB), and runs all four diffusion passes as
fp8 DoubleRow matmuls (2 packed contraction rows/cycle). Diffused features
are small contributors to the output (the graph averages 4000 nodes), so fp8
error lands ~1e-4 relative; order-k features carry power-of-2 scales folded
into the PSUM-evacuation copies and undone by host-side weight pre-scaling.

Layouts per core: activations node-major fp8 [128p x (chunk, col)] for
diffusion; gate/candidate convs run fp16 from per-band staging tiles
(PE transposes for diffused features, XBAR DMA-transpose from DRAM for the
raw [x;h] features). Gate/candidate nonlinearities on ACT, elementwise on
DVE, combine fused into the last diffusion's band loop.
"""

import numpy as np
import ml_dtypes

import concourse.bass as bass
from concourse import bacc
import concourse.mybir as mybir
import concourse.tile as tile
from concourse.bass_utils import run_bass_kernel_spmd

# problem constants
B, D_IN, D_H, NN = 16, 32, 64, 4000
NCORES = 8
B_LOC = B // NCORES          # batches per core
C = D_IN + D_H               # 96 channels into each gate conv
BC = B_LOC * C               # node-major column count (b-major: [b0 c96 | b1 c96])
BH = B_LOC * D_H             # stacked batch-hidden rows (128)
NP = 4096                    # contraction node dim padded to 32 chunks
CHUNK = 128
NCH = NP // CHUNK            # 32 contraction chunks
NPR = NCH // 2               # 16 DoubleRow chunk pairs
NBAND = 8                    # output-node bands: 7x512 + 416 (= 4000, no pad)
BW = [512] * 7 + [416]
BOFF = [512 * g for g in range(NBAND)]
AOFF = [NCH * 512 * g for g in range(NBAND)]   # at_d col offset per band

F8 = mybir.dt.float8e4
F16 = mybir.dt.float16
F32 = mybir.dt.float32
DR = mybir.MatmulPerfMode.DoubleRow
E4NP = ml_dtypes.float8_e4m3

# fp8 scale chain: adjacency carries x4096 (2^12).
#   z1T carries x32   -> evac scale 32/4096
#   z2T carries x512  -> evac scale 512/(4096*32)
#   zc1 carries x64   -> evac scale 64/4096
#   zc2 stage x512    -> evac scale 512/(4096*64)
S_Z1E, S_Z2E = 2.0 ** -7, 2.0 ** -8
S_C1E, S_C2E = 2.0 ** -6, 2.0 ** -9
# matching host-side weight descales: gate W1 /32, W2 /512; cand x-part
# W1 /32, W2 /512; cand rh-part W1 /64, W2 /512.


def _mlist(g):
    """(offset, width) of the 128-wide m-chunks inside band g."""
    w = BW[g]
    out = []
    mo = 0
    while mo < w:
        out.append((mo, min(CHUNK, w - mo)))
        mo += CHUNK
    return out


def build_program():
    nc = bacc.Bacc("TRN2", target_bir_lowering=False, debug=False)

    at_d = nc.dram_tensor("at", [CHUNK, NCH * NN], F8, kind="ExternalInput").ap()
    zt_d = nc.dram_tensor("zt", [CHUNK, NCH * BC], F8, kind="ExternalInput").ap()
    # node-major [x;h] fp16, padded to 128 cols/batch for XBAR dma transpose
    zn_d = nc.dram_tensor("zn", [NP, B_LOC, CHUNK], F16, kind="ExternalInput").ap()
    h_d = nc.dram_tensor("h", [BH, NN], F16, kind="ExternalInput").ap()
    wf_d = nc.dram_tensor("wf", [3, C, D_H], F16, kind="ExternalInput").ap()
    wu_d = nc.dram_tensor("wu", [3, C, D_H], F16, kind="ExternalInput").ap()
    wcx_d = nc.dram_tensor("wcx", [3, D_IN, D_H], F16, kind="ExternalInput").ap()
    wcrh_d = nc.dram_tensor("wcrh", [3, D_H, D_H], F16, kind="ExternalInput").ap()
    bf_d = nc.dram_tensor("bf", [BH, 1], F32, kind="ExternalInput").ap()
    bu_d = nc.dram_tensor("bu", [BH, 1], F32, kind="ExternalInput").ap()
    bc_d = nc.dram_tensor("bcb", [BH, 1], F32, kind="ExternalInput").ap()
    id16_d = nc.dram_tensor("id16", [CHUNK, CHUNK], F16, kind="ExternalInput").ap()
    id8_d = nc.dram_tensor("id8", [CHUNK, CHUNK], F8, kind="ExternalInput").ap()
    out_d = nc.dram_tensor("out", [B_LOC, D_H, NN], F32, kind="ExternalOutput").ap()

    with tile.TileContext(nc) as tc:
        _body(tc, locals())
    nc.compile()
    return nc


def _body(tc, aps):
    nc = tc.nc
    at_d, zt_d, zn_d, h_d = aps["at_d"], aps["zt_d"], aps["zn_d"], aps["h_d"]
    wf_d, wu_d, wcx_d, wcrh_d = (
        aps["wf_d"], aps["wu_d"], aps["wcx_d"], aps["wcrh_d"])
    bf_d, bu_d, bc_d = aps["bf_d"], aps["bu_d"], aps["bc_d"]
    id16_d, id8_d, out_d = aps["id16_d"], aps["id8_d"], aps["out_d"]

    SIG = mybir.ActivationFunctionType.Sigmoid
    TANH = mybir.ActivationFunctionType.Tanh
    COPY = mybir.ActivationFunctionType.Copy

    with (
        tc.tile_pool(name="const", bufs=1) as cpool,
        tc.tile_pool(name="amat", bufs=1) as apool,       # resident adjacency
        tc.tile_pool(name="nm8", bufs=2) as nmpool,       # rotating node-major fp8
        tc.tile_pool(name="perst", bufs=1) as ppool,
        tc.tile_pool(name="stageA", bufs=2) as sApool,    # conv feature stages
        tc.tile_pool(name="stageB", bufs=2) as sBpool,    # wide f16 stages
        tc.tile_pool(name="stageC", bufs=2) as sCpool,    # f32 combine stages
        tc.tile_pool(name="psum", bufs=8, space="PSUM") as pspool,
    ):
        # ---- persistent loads ----
        # DMA priority: phase 1 is gated on ztT + at0, so those go first on
        # separate rings; weights/h/idm are not needed until phase 2.
        # small loads first — they must not queue behind the ring-throttled
        # adjacency triggers
        idm = cpool.tile([CHUNK, CHUNK], F16, tag="idm")
        nc.sync.dma_start(out=idm[:], in_=id16_d[:])
        idm8 = cpool.tile([CHUNK, CHUNK], F8, tag="idm8")
        nc.sync.dma_start(out=idm8[:], in_=id8_d[:])
        bf_sb = cpool.tile([BH, 1], F32, tag="bf")
        nc.sync.dma_start(out=bf_sb[:], in_=bf_d[:])
        bu_sb = cpool.tile([BH, 1], F32, tag="bu")
        nc.sync.dma_start(out=bu_sb[:], in_=bu_d[:])
        bc_sb = cpool.tile([BH, 1], F32, tag="bc")
        nc.sync.dma_start(out=bc_sb[:], in_=bc_d[:])
        wf_sb = [cpool.tile([C, D_H], F16, tag=f"wf{k}", name=f"wf{k}")
                 for k in range(3)]
        wu_sb = [cpool.tile([C, D_H], F16, tag=f"wu{k}", name=f"wu{k}")
                 for k in range(3)]
        wcx_sb = [cpool.tile([D_IN, D_H], F16, tag=f"wcx{k}", name=f"wcx{k}")
                  for k in range(3)]
        wcrh_sb = [cpool.tile([D_H, D_H], F16, tag=f"wcrh{k}", name=f"wcrh{k}")
                   for k in range(3)]
        for k in range(3):
            nc.scalar.dma_start(out=wf_sb[k][:], in_=wf_d[k])
            nc.scalar.dma_start(out=wu_sb[k][:], in_=wu_d[k])
            nc.scalar.dma_start(out=wcx_sb[k][:], in_=wcx_d[k])
            nc.scalar.dma_start(out=wcrh_sb[k][:], in_=wcrh_d[k])

        ztT = nmpool.tile([CHUNK, NCH * BC], F8, tag="nm", name="ztT")
        nc.sync.dma_start(out=ztT[:, :], in_=zt_d[:, :])
        # resident adjacency^T (x4096, fp8), one tile per output band;
        # each band split across both HWDGE rings so it lands in ~4us
        at_sb = []
        for g in range(NBAND):
            t = apool.tile([CHUNK, NCH * BW[g]], F8, tag=f"at{g}",
                           name=f"at{g}")
            half = (NCH // 2) * BW[g]
            nc.sync.dma_start(out=t[:, 0:half],
                              in_=at_d[:, AOFF[g]:AOFF[g] + half])
            nc.scalar.dma_start(
                out=t[:, half:NCH * BW[g]],
                in_=at_d[:, AOFF[g] + half:AOFF[g] + NCH * BW[g]])
            at_sb.append(t[:, :].rearrange("p (j m) -> p j m", j=NCH))

        # h is not needed until the phase-2 gate math (~60us in)
        h_st = ppool.tile([BH, NN], F16, tag="h_st")
        nc.scalar.dma_start(out=h_st[:], in_=h_d[:])

        u_st = ppool.tile([BH, NN], F16, tag="u_st")
        rh_st = ppool.tile([BH, NP], F16, tag="rh_st")
        nc.vector.memset(rh_st[:, NN:NP], 0.0)
        c_part = ppool.tile([BH, NN], F16, tag="c_part")
        rhT = ppool.tile([CHUNK, NCH * BH], F8, tag="rhT")
        zc1_bm = ppool.tile([BH, NP], F8, tag="zc1_bm")
        nc.vector.memset(zc1_bm[:, NN:NP], 0.0)
        zc1T = ppool.tile([CHUNK, NCH * BH], F8, tag="zc1T")

        zt3 = ztT[:, :].rearrange("p (j f) -> p j f", j=NCH)
        rhT3 = rhT[:, :].rearrange("p (j f) -> p j f", j=NCH)
        zc1T3 = zc1T[:, :].rearrange("p (j f) -> p j f", j=NCH)

        def sa_band(g, src3, dst3, evac_scale):
            """band g of dst = A @ src, node-major -> node-major."""
            ml = _mlist(g)
            pss = [pspool.tile([CHUNK, BC], F32, tag="ps", name=f"psd{mi}")
                   for mi in range(len(ml))]
            for jj in range(NPR):
                for mi, (mo, mw) in enumerate(ml):
                    nc.tensor.matmul(
                        pss[mi][0:mw, :],
                        lhsT=at_sb[g][:, 2 * jj:2 * jj + 2, mo:mo + mw],
                        rhs=src3[:, 2 * jj:2 * jj + 2, :],
                        start=(jj == 0), stop=(jj == NPR - 1), perf_mode=DR)
            for mi, (mo, mw) in enumerate(ml):
                # evac on DVE: the ACT queue holds the ring-throttled
                # adjacency DMA triggers early on and must not gate PSUM reuse
                nc.vector.tensor_scalar_mul(
                    out=dst3[0:mw, g * 4 + mi, :], in0=pss[mi][0:mw, :],
                    scalar1=evac_scale)

        # ---- phase 1: z1 = A z ----
        z1T = nmpool.tile([CHUNK, NCH * BC], F8, tag="nm", name="z1T")
        z13 = z1T[:, :].rearrange("p (j f) -> p j f", j=NCH)
        nc.vector.memset(z13[:, NCH - 1, :], 0.0)
        for g in range(NBAND):
            sa_band(g, zt3, z13, S_Z1E)

        # ---- phase 2: z2 = A z1, fused with gate convs, rh, rhT ----
        z2T = nmpool.tile([CHUNK, NCH * BC], F8, tag="nm", name="z2T")
        z23 = z2T[:, :].rearrange("p (j f) -> p j f", j=NCH)
        nc.vector.memset(z23[:, NCH - 1, :], 0.0)

        def post2(g):
            ml = _mlist(g)
            m0, w = BOFF[g], BW[g]
            for b in range(B_LOC):
                rows = slice(b * D_H, (b + 1) * D_H)
                # stage conv features (fp16, base partition 0)
                z0s = sBpool.tile([CHUNK, 512], F16, tag="z0s", name="z0s")
                nc.sync.dma_start_transpose(
                    out=z0s[:, 0:w], in_=zn_d[m0:m0 + w, b, :])
                z1s = sApool.tile([C, 512], F16, tag="z1s", name="z1s")
                z2s = sApool.tile([C, 512], F16, tag="z2s", name="z2s")
                for src3, dst in ((z13, z1s), (z23, z2s)):
                    for mi, (mo, mw) in enumerate(ml):
                        # fp8 PE transpose writes PSUM at element step 2
                        pt = pspool.tile([C, 2 * CHUNK], F8, tag="ps",
                                         name="pt")
                        nc.tensor.transpose(
                            pt[:, 0:2 * CHUNK:2],
                            src3[:, g * 4 + mi, b * C:(b + 1) * C],
                            idm8[:, :])
                        nc.vector.tensor_copy(out=dst[:, mo:mo + mw],
                                              in_=pt[:, 0:2 * mw:2])
                feats = (z0s[0:C, 0:w], z1s[:, 0:w], z2s[:, 0:w])
                feats_x = (z0s[0:D_IN, 0:w], z1s[0:D_IN, 0:w],
                           z2s[0:D_IN, 0:w])
                psf = pspool.tile([BH, 512], F32, tag="ps", name="psf") \
                    if b == 0 else psf
                psu = pspool.tile([BH, 512], F32, tag="ps", name="psu") \
                    if b == 0 else psu
                psx = pspool.tile([BH, 512], F32, tag="ps", name="psx") \
                    if b == 0 else psx
                for k in range(3):
                    nc.tensor.matmul(psf[rows, 0:w], lhsT=wf_sb[k],
                                     rhs=feats[k], start=(k == 0),
                                     stop=(k == 2))
                for k in range(3):
                    nc.tensor.matmul(psu[rows, 0:w], lhsT=wu_sb[k],
                                     rhs=feats[k], start=(k == 0),
                                     stop=(k == 2))
                for k in range(3):
                    nc.tensor.matmul(psx[rows, 0:w], lhsT=wcx_sb[k],
                                     rhs=feats_x[k], start=(k == 0),
                                     stop=(k == 2))
            # gate nonlinearities + rh, full 128 partitions
            rst = sBpool.tile([BH, 512], F16, tag="rst", name="rst")
            nc.scalar.activation(rst[:, 0:w], psf[:, 0:w], SIG, bias=bf_sb[:, :])
            nc.scalar.activation(u_st[:, m0:m0 + w], psu[:, 0:w], SIG,
                                 bias=bu_sb[:, :])
            nc.gpsimd.tensor_mul(out=rh_st[:, m0:m0 + w], in0=rst[:, 0:w],
                                 in1=h_st[:, m0:m0 + w])
            nc.vector.tensor_copy(out=c_part[:, m0:m0 + w], in_=psx[:, 0:w])
            # rhT for the candidate diffusion (node-major fp8)
            for b in range(B_LOC):
                rows = slice(b * D_H, (b + 1) * D_H)
                for mi, (mo, mw) in enumerate(ml):
                    ch = g * 4 + mi
                    ptr = pspool.tile([CHUNK, D_H], F16, tag="ps", name="ptr")
                    nc.tensor.transpose(
                        ptr[:, :],
                        rh_st[rows, ch * CHUNK:(ch + 1) * CHUNK],
                        idm[rows, rows])
                    nc.vector.tensor_copy(
                        out=rhT3[:, ch, b * D_H:(b + 1) * D_H], in_=ptr[:, :])

        # phase 2 driver: band g's dependent work staggered one band behind
        # the sa matmuls so the ACT/DVE round trips hide under PE work
        for g in range(NBAND):
            sa_band(g, z13, z23, S_Z2E)
            if g > 0:
                post2(g - 1)
        post2(NBAND - 1)

        # ---- phase 3: zc1 = A rh (activations stationary, adj moving) ----
        def mm_sz(g, lhsT3, name):
            psc = pspool.tile([BH, 512], F32, tag="ps", name=name)
            w = BW[g]
            for jj in range(NPR):
                nc.tensor.matmul(
                    psc[:, 0:w],
                    lhsT=lhsT3[:, 2 * jj:2 * jj + 2, :],
                    rhs=at_sb[g][:, 2 * jj:2 * jj + 2, 0:w],
                    start=(jj == 0), stop=(jj == NPR - 1), perf_mode=DR)
            return psc

        def post3(g):
            for b in range(B_LOC):
                rows = slice(b * D_H, (b + 1) * D_H)
                for mi, (mo, mw) in enumerate(_mlist(g)):
                    ch = g * 4 + mi
                    ptc = pspool.tile([CHUNK, 2 * D_H], F8, tag="ps",
                                      name="ptc")
                    nc.tensor.transpose(
                        ptc[:, 0:2 * D_H:2],
                        zc1_bm[rows, ch * CHUNK:(ch + 1) * CHUNK],
                        idm8[rows, rows])
                    nc.vector.tensor_copy(
                        out=zc1T3[:, ch, b * D_H:(b + 1) * D_H],
                        in_=ptc[:, 0:2 * D_H:2])

        for g in range(NBAND):
            m0, w = BOFF[g], BW[g]
            psc = mm_sz(g, rhT3, "psc")
            nc.scalar.activation(zc1_bm[:, m0:m0 + w], psc[:, 0:w], COPY,
                                 scale=S_C1E)
            if g > 0:
                post3(g - 1)
        post3(NBAND - 1)

        # ---- phase 4: zc2 = A zc1, fused candidate conv + combine ----
        def cons4(g, psc2):
            m0, w = BOFF[g], BW[g]
            zc2s = sBpool.tile([BH, 512], F16, tag="zc2s", name="zc2s")
            nc.scalar.activation(zc2s[:, 0:w], psc2[:, 0:w], COPY,
                                 scale=S_C2E)
            zc1s = sBpool.tile([BH, 512], F16, tag="zc1s", name="zc1s")
            nc.vector.tensor_copy(out=zc1s[:, 0:w], in_=zc1_bm[:, m0:m0 + w])
            # batch-1 features need base partition 0: SBUF->SBUF DMA restage
            b1rh = sApool.tile([D_H, 512], F16, tag="b1rh", name="b1rh")
            nc.scalar.dma_start(out=b1rh[:, 0:w], in_=rh_st[D_H:BH, m0:m0 + w])
            b1c1 = sApool.tile([D_H, 512], F16, tag="b1c1", name="b1c1")
            nc.scalar.dma_start(out=b1c1[:, 0:w], in_=zc1s[D_H:BH, 0:w])
            b1c2 = sApool.tile([D_H, 512], F16, tag="b1c2", name="b1c2")
            nc.scalar.dma_start(out=b1c2[:, 0:w], in_=zc2s[D_H:BH, 0:w])
            # combine in 256-col chunks so the 4-engine chain pipelines
            for c0 in range(0, w, 256):
                cw = min(256, w - c0)
                n0 = m0 + c0
                psc3 = pspool.tile([BH, 256], F32, tag="ps", name="psc3")
                for b in range(B_LOC):
                    rows = slice(b * D_H, (b + 1) * D_H)
                    terms = ((rh_st[0:D_H, n0:n0 + cw],
                              zc1s[0:D_H, c0:c0 + cw],
                              zc2s[0:D_H, c0:c0 + cw]) if b == 0 else
                             (b1rh[:, c0:c0 + cw], b1c1[:, c0:c0 + cw],
                              b1c2[:, c0:c0 + cw]))
                    for k in range(3):
                        nc.tensor.matmul(psc3[rows, 0:cw], lhsT=wcrh_sb[k],
                                         rhs=terms[k], start=(k == 0),
                                         stop=(k == 2))
                tt = sCpool.tile([BH, 256], F16, tag="tt", name="tt")
                nc.vector.tensor_add(out=tt[:, 0:cw], in0=psc3[:, 0:cw],
                                     in1=c_part[:, n0:n0 + cw])
                cst = sCpool.tile([BH, 256], F32, tag="cst", name="cst")
                nc.scalar.activation(cst[:, 0:cw], tt[:, 0:cw], TANH,
                                     bias=bc_sb[:, :])
                # combine on the otherwise-idle Pool engine; tt holds h-c
                nc.gpsimd.tensor_sub(out=tt[:, 0:cw], in0=h_st[:, n0:n0 + cw],
                                     in1=cst[:, 0:cw])
                nc.gpsimd.tensor_mul(out=tt[:, 0:cw], in0=u_st[:, n0:n0 + cw],
                                     in1=tt[:, 0:cw])
                nc.gpsimd.tensor_add(out=cst[:, 0:cw], in0=cst[:, 0:cw],
                                     in1=tt[:, 0:cw])
                for b in range(B_LOC):
                    nc.sync.dma_start(
                        out=out_d[b][:, n0:n0 + cw],
                        in_=cst[b * D_H:(b + 1) * D_H, 0:cw])

        psc2_prev = None
        for g in range(NBAND):
            psc2 = mm_sz(g, zc1T3, "psc2")
            if g > 0:
                cons4(g - 1, psc2_prev)
            psc2_prev = psc2
        cons4(NBAND - 1, psc2_prev)


# ---- host-side driver ----
_CACHED_NC = None
TRACE = False           # set True (e.g. from test.py) to capture an NTFF profile
TRACE_DIR = None
LAST_RESULTS = None     # BassKernelResults of the most recent kernel() call


def _host_prep(x, h, adj, Wf, bf, Wu, bu, Wc, bc):
    """Shard + cast + layout inputs for the 8 cores. Returns list of in_maps."""
    atp = np.zeros((NP, NN), dtype=np.float32)
    atp[:NN] = adj.T * 4096.0
    at8 = atp.astype(E4NP)                       # [4096, 4000]
    blocks = at8.reshape(NCH, CHUNK, NN)
    cols = [np.ascontiguousarray(
        blocks[:, :, BOFF[g]:BOFF[g] + BW[g]].transpose(1, 0, 2)
    ).reshape(CHUNK, NCH * BW[g]) for g in range(NBAND)]
    at_h = np.ascontiguousarray(np.concatenate(cols, axis=1))

    id16 = np.eye(CHUNK, dtype=np.float16)
    id8 = np.eye(CHUNK, dtype=E4NP)

    wsc = {"wf": (1.0, 1 / 32., 1 / 512.), "wu": (1.0, 1 / 32., 1 / 512.),
           "wcx": (1.0, 1 / 32., 1 / 512.), "wcrh": (1.0, 1 / 64., 1 / 512.)}

    def wsplit(W, key, lo, hi):
        return np.ascontiguousarray(np.stack(
            [(W[:, k * C + lo:k * C + hi].T * wsc[key][k]).astype(np.float16)
             for k in range(3)]))

    def bstack(v):
        return np.concatenate([v] * B_LOC).reshape(BH, 1).astype(np.float32)

    shared = {
        "at": at_h, "id16": id16, "id8": id8,
        "wf": wsplit(Wf, "wf", 0, C), "wu": wsplit(Wu, "wu", 0, C),
        "wcx": wsplit(Wc, "wcx", 0, D_IN), "wcrh": wsplit(Wc, "wcrh", D_IN, C),
        "bf": bstack(bf), "bu": bstack(bu), "bcb": bstack(bc),
    }
    in_maps = []
    for core in range(NCORES):
        bs = slice(core * B_LOC, (core + 1) * B_LOC)
        z = np.concatenate([x[bs], h[bs]], axis=1)       # [B_LOC, C, NN]
        znm = z.transpose(2, 0, 1)                       # [NN, B_LOC, C]
        ztp = np.zeros((NP, BC), dtype=np.float32)
        ztp[:NN] = znm.reshape(NN, BC)
        zt8 = np.ascontiguousarray(
            ztp.astype(E4NP).reshape(NCH, CHUNK, BC).transpose(1, 0, 2)
        ).reshape(CHUNK, NCH * BC)
        znp = np.zeros((NP, B_LOC, CHUNK), dtype=np.float16)
        znp[:NN, :, :C] = znm
        h_p = np.ascontiguousarray(
            h[bs].astype(np.float16).reshape(BH, NN))
        in_maps.append(dict(shared, zt=zt8, zn=znp, h=h_p))
    return in_maps


def kernel(**inputs):
    global _CACHED_NC, LAST_RESULTS
    inputs = {k: np.asarray(v) for k, v in inputs.items()}
    if _CACHED_NC is None:
        _CACHED_NC = build_program()
    in_maps = _host_prep(**inputs)
    kw = {}
    if TRACE:
        kw = dict(trace=True, tmpdir=TRACE_DIR)
    res = run_bass_kernel_spmd(_CACHED_NC, in_maps,
                               core_ids=list(range(NCORES)), **kw)
    LAST_RESULTS = res
    outs = [res.results[i]["out"] for i in range(NCORES)]
    return np.concatenate(outs, axis=0).astype(np.float32)


if __name__ == "__main__":
    rng = np.random.default_rng(0)
    ins = {
        "x": rng.standard_normal((B, D_IN, NN), dtype=np.float32),
        "h": rng.standard_normal((B, D_H, NN), dtype=np.float32),
        "adj": rng.random((NN, NN), dtype=np.float32) / NN,
        "Wf": rng.standard_normal((D_H, 3 * C), dtype=np.float32) * 0.05,
        "Wu": rng.standard_normal((D_H, 3 * C), dtype=np.float32) * 0.05,
        "Wc": rng.standard_normal((D_H, 3 * C), dtype=np.float32) * 0.05,
        "bf": rng.standard_normal(D_H).astype(np.float32) * 0.05,
        "bu": rng.standard_normal(D_H).astype(np.float32) * 0.05,
        "bc": rng.standard_normal(D_H).astype(np.float32) * 0.05,
    }
    out = kernel(**ins)
    print(out.shape, out.dtype)


# revision 27
# speedup vs baseline: 1.1301x; 1.0287x over previous
"""GCGRU cell (order-2 graph diffusion GRU) Trainium2 Bass kernel, v2.

Strategy: data-parallel over batch (B=16 -> 2 batches per core x 8 cores).
The dominant cost in v1 was streaming the 32MB fp16 adjacency from HBM four
times per core (DMA 99% busy). v2 keeps the whole adjacency RESIDENT in SBUF
as fp8 (x4096 pre-scale keeps the row-normalized values out of e4m3's
denormal range), loaded once (~16MB), and runs all four diffusion passes as
fp8 DoubleRow matmuls (2 packed contraction rows/cycle). Diffused features
are small contributors to the output (the graph averages 4000 nodes), so fp8
error lands ~1e-4 relative; order-k features carry power-of-2 scales folded
into the PSUM-evacuation copies and undone by host-side weight pre-scaling.

Layouts per core: activations node-major fp8 [128p x (chunk, col)] for
diffusion; gate/candidate convs run fp16 from per-band staging tiles
(PE transposes for diffused features, XBAR DMA-transpose from DRAM for the
raw [x;h] features). Gate/candidate nonlinearities on ACT, elementwise on
DVE, combine fused into the last diffusion's band loop.
"""

import numpy as np
import ml_dtypes

import concourse.bass as bass
from concourse import bacc
import concourse.mybir as mybir
import concourse.tile as tile
from concourse.bass_utils import run_bass_kernel_spmd

# problem constants
B, D_IN, D_H, NN = 16, 32, 64, 4000
NCORES = 8
B_LOC = B // NCORES          # batches per core
C = D_IN + D_H               # 96 channels into each gate conv
BC = B_LOC * C               # node-major column count (b-major: [b0 c96 | b1 c96])
BH = B_LOC * D_H             # stacked batch-hidden rows (128)
NP = 4096                    # contraction node dim padded to 32 chunks
CHUNK = 128
NCH = NP // CHUNK            # 32 contraction chunks
NPR = NCH // 2               # 16 DoubleRow chunk pairs
NBAND = 8                    # output-node bands: 7x512 + 416 (= 4000, no pad)
BW = [512] * 7 + [416]
BOFF = [512 * g for g in range(NBAND)]
AOFF = [NCH * 512 * g for g in range(NBAND)]   # at_d col offset per band

F8 = mybir.dt.float8e4
F16 = mybir.dt.float16
F32 = mybir.dt.float32
DR = mybir.MatmulPerfMode.DoubleRow
E4NP = ml_dtypes.float8_e4m3

# fp8 scale chain: adjacency carries x4096 (2^12).
#   z1T carries x32   -> evac scale 32/4096
#   z2T carries x512  -> evac scale 512/(4096*32)
#   zc1 carries x64   -> evac scale 64/4096
#   zc2 stage x512    -> evac scale 512/(4096*64)
S_Z1E, S_Z2E = 2.0 ** -7, 2.0 ** -8
S_C1E, S_C2E = 2.0 ** -6, 2.0 ** -9
# matching host-side weight descales: gate W1 /32, W2 /512; cand x-part
# W1 /32, W2 /512; cand rh-part W1 /64, W2 /512.


def _mlist(g):
    """(offset, width) of the 128-wide m-chunks inside band g."""
    w = BW[g]
    out = []
    mo = 0
    while mo < w:
        out.append((mo, min(CHUNK, w - mo)))
        mo += CHUNK
    return out


def build_program():
    nc = bacc.Bacc("TRN2", target_bir_lowering=False, debug=False)

    at_d = nc.dram_tensor("at", [CHUNK, NCH * NN], F8, kind="ExternalInput").ap()
    zt_d = nc.dram_tensor("zt", [CHUNK, NCH * BC], F8, kind="ExternalInput").ap()
    # node-major [x;h] fp16, padded to 128 cols/batch for XBAR dma transpose
    zn_d = nc.dram_tensor("zn", [NP, B_LOC, CHUNK], F16, kind="ExternalInput").ap()
    h_d = nc.dram_tensor("h", [BH, NN], F16, kind="ExternalInput").ap()
    wf_d = nc.dram_tensor("wf", [3, C, D_H], F16, kind="ExternalInput").ap()
    wu_d = nc.dram_tensor("wu", [3, C, D_H], F16, kind="ExternalInput").ap()
    wcx_d = nc.dram_tensor("wcx", [3, D_IN, D_H], F16, kind="ExternalInput").ap()
    wcrh_d = nc.dram_tensor("wcrh", [3, D_H, D_H], F16, kind="ExternalInput").ap()
    bf_d = nc.dram_tensor("bf", [BH, 1], F32, kind="ExternalInput").ap()
    bu_d = nc.dram_tensor("bu", [BH, 1], F32, kind="ExternalInput").ap()
    bc_d = nc.dram_tensor("bcb", [BH, 1], F32, kind="ExternalInput").ap()
    id16_d = nc.dram_tensor("id16", [CHUNK, CHUNK], F16, kind="ExternalInput").ap()
    id8_d = nc.dram_tensor("id8", [CHUNK, CHUNK], F8, kind="ExternalInput").ap()
    out_d = nc.dram_tensor("out", [B_LOC, D_H, NN], F32, kind="ExternalOutput").ap()

    with tile.TileContext(nc) as tc:
        _body(tc, locals())
    nc.compile()
    return nc


def _body(tc, aps):
    nc = tc.nc
    at_d, zt_d, zn_d, h_d = aps["at_d"], aps["zt_d"], aps["zn_d"], aps["h_d"]
    wf_d, wu_d, wcx_d, wcrh_d = (
        aps["wf_d"], aps["wu_d"], aps["wcx_d"], aps["wcrh_d"])
    bf_d, bu_d, bc_d = aps["bf_d"], aps["bu_d"], aps["bc_d"]
    id16_d, id8_d, out_d = aps["id16_d"], aps["id8_d"], aps["out_d"]

    SIG = mybir.ActivationFunctionType.Sigmoid
    TANH = mybir.ActivationFunctionType.Tanh
    COPY = mybir.ActivationFunctionType.Copy

    with (
        tc.tile_pool(name="const", bufs=1) as cpool,
        tc.tile_pool(name="amat", bufs=1) as apool,       # resident adjacency
        tc.tile_pool(name="nm8", bufs=2) as nmpool,       # rotating node-major fp8
        tc.tile_pool(name="perst", bufs=1) as ppool,
        tc.tile_pool(name="stageA", bufs=2) as sApool,    # conv feature stages
        tc.tile_pool(name="stageB", bufs=2) as sBpool,    # wide f16 stages
        tc.tile_pool(name="stageC", bufs=2) as sCpool,    # f32 combine stages
        tc.tile_pool(name="psum", bufs=8, space="PSUM") as pspool,
    ):
        # ---- persistent loads ----
        # DMA priority: phase 1 is gated on ztT + at0, so those go first on
        # separate rings; weights/h/idm are not needed until phase 2.
        # small loads first — they must not queue behind the ring-throttled
        # adjacency triggers
        idm = cpool.tile([CHUNK, CHUNK], F16, tag="idm")
        nc.sync.dma_start(out=idm[:], in_=id16_d[:])
        idm8 = cpool.tile([CHUNK, CHUNK], F8, tag="idm8")
        nc.sync.dma_start(out=idm8[:], in_=id8_d[:])
        bf_sb = cpool.tile([BH, 1], F32, tag="bf")
        nc.sync.dma_start(out=bf_sb[:], in_=bf_d[:])
        bu_sb = cpool.tile([BH, 1], F32, tag="bu")
        nc.sync.dma_start(out=bu_sb[:], in_=bu_d[:])
        bc_sb = cpool.tile([BH, 1], F32, tag="bc")
        nc.sync.dma_start(out=bc_sb[:], in_=bc_d[:])
        wf_sb = [cpool.tile([C, D_H], F16, tag=f"wf{k}", name=f"wf{k}")
                 for k in range(3)]
        wu_sb = [cpool.tile([C, D_H], F16, tag=f"wu{k}", name=f"wu{k}")
                 for k in range(3)]
        wcx_sb = [cpool.tile([D_IN, D_H], F16, tag=f"wcx{k}", name=f"wcx{k}")
                  for k in range(3)]
        wcrh_sb = [cpool.tile([D_H, D_H], F16, tag=f"wcrh{k}", name=f"wcrh{k}")
                   for k in range(3)]
        for k in range(3):
            nc.scalar.dma_start(out=wf_sb[k][:], in_=wf_d[k])
            nc.scalar.dma_start(out=wu_sb[k][:], in_=wu_d[k])
            nc.scalar.dma_start(out=wcx_sb[k][:], in_=wcx_d[k])
            nc.scalar.dma_start(out=wcrh_sb[k][:], in_=wcrh_d[k])

        ztT = nmpool.tile([CHUNK, NCH * BC], F8, tag="nm", name="ztT")
        nc.scalar.dma_start(out=ztT[:, :], in_=zt_d[:, :])
        # resident adjacency^T (x4096, fp8), one tile per output band.
        # Band 0 is latency-critical: split across both rings. Later bands
        # alternate whole-band per ring (trigger-instruction throughput is
        # the binding resource, not HBM bandwidth).
        at_sb = []
        at_t = []
        for g in range(NBAND):
            t = apool.tile([CHUNK, NCH * BW[g]], F8, tag=f"at{g}",
                           name=f"at{g}")
            at_t.append(t)
            at_sb.append(t[:, :].rearrange("p (j m) -> p j m", j=NCH))
        half = (NCH // 2) * BW[0]
        nc.sync.dma_start(out=at_t[0][:, 0:half], in_=at_d[:, 0:half])
        nc.scalar.dma_start(out=at_t[0][:, half:NCH * BW[0]],
                            in_=at_d[:, half:NCH * BW[0]])
        for g in range(1, NBAND):
            eng = nc.scalar if g % 2 == 1 else nc.sync
            eng.dma_start(out=at_t[g][:],
                          in_=at_d[:, AOFF[g]:AOFF[g] + NCH * BW[g]])

        # h is not needed until the phase-2 gate math (~60us in)
        h_st = ppool.tile([BH, NN], F16, tag="h_st")
        nc.scalar.dma_start(out=h_st[:], in_=h_d[:])

        u_st = ppool.tile([BH, NN], F16, tag="u_st")
        rh_st = ppool.tile([BH, NP], F16, tag="rh_st")
        nc.vector.memset(rh_st[:, NN:NP], 0.0)
        c_part = ppool.tile([BH, NN], F16, tag="c_part")
        rhT = ppool.tile([CHUNK, NCH * BH], F8, tag="rhT")
        zc1_bm = ppool.tile([BH, NP], F8, tag="zc1_bm")
        nc.vector.memset(zc1_bm[:, NN:NP], 0.0)
        zc1T = ppool.tile([CHUNK, NCH * BH], F8, tag="zc1T")

        zt3 = ztT[:, :].rearrange("p (j f) -> p j f", j=NCH)
        rhT3 = rhT[:, :].rearrange("p (j f) -> p j f", j=NCH)
        zc1T3 = zc1T[:, :].rearrange("p (j f) -> p j f", j=NCH)

        def sa_band(g, src3, dst3, evac_scale):
            """band g of dst = A @ src, node-major -> node-major."""
            ml = _mlist(g)
            pss = [pspool.tile([CHUNK, BC], F32, tag="ps", name=f"psd{mi}")
                   for mi in range(len(ml))]
            for jj in range(NPR):
                for mi, (mo, mw) in enumerate(ml):
                    nc.tensor.matmul(
                        pss[mi][0:mw, :],
                        lhsT=at_sb[g][:, 2 * jj:2 * jj + 2, mo:mo + mw],
                        rhs=src3[:, 2 * jj:2 * jj + 2, :],
                        start=(jj == 0), stop=(jj == NPR - 1), perf_mode=DR)
            for mi, (mo, mw) in enumerate(ml):
                # evac on DVE: the ACT queue holds the ring-throttled
                # adjacency DMA triggers early on and must not gate PSUM reuse
                nc.vector.tensor_scalar_mul(
                    out=dst3[0:mw, g * 4 + mi, :], in0=pss[mi][0:mw, :],
                    scalar1=evac_scale)

        # ---- phase 1: z1 = A z ----
        z1T = nmpool.tile([CHUNK, NCH * BC], F8, tag="nm", name="z1T")
        z13 = z1T[:, :].rearrange("p (j f) -> p j f", j=NCH)
        nc.vector.memset(z13[:, NCH - 1, :], 0.0)
        for g in range(NBAND):
            sa_band(g, zt3, z13, S_Z1E)

        # ---- phase 2: z2 = A z1, fused with gate convs, rh, rhT ----
        z2T = nmpool.tile([CHUNK, NCH * BC], F8, tag="nm", name="z2T")
        z23 = z2T[:, :].rearrange("p (j f) -> p j f", j=NCH)
        nc.vector.memset(z23[:, NCH - 1, :], 0.0)

        def post2(g):
            ml = _mlist(g)
            m0, w = BOFF[g], BW[g]
            for b in range(B_LOC):
                rows = slice(b * D_H, (b + 1) * D_H)
                # stage conv features (fp16, base partition 0)
                z0s = sBpool.tile([CHUNK, 512], F16, tag="z0s", name="z0s")
                nc.sync.dma_start_transpose(
                    out=z0s[:, 0:w], in_=zn_d[m0:m0 + w, b, :])
                z1s = sApool.tile([C, 512], F16, tag="z1s", name="z1s")
                z2s = sApool.tile([C, 512], F16, tag="z2s", name="z2s")
                for src3, dst in ((z13, z1s), (z23, z2s)):
                    for mi, (mo, mw) in enumerate(ml):
                        # fp8 PE transpose writes PSUM at element step 2
                        pt = pspool.tile([C, 2 * CHUNK], F8, tag="ps",
                                         name="pt")
                        nc.tensor.transpose(
                            pt[:, 0:2 * CHUNK:2],
                            src3[:, g * 4 + mi, b * C:(b + 1) * C],
                            idm8[:, :])
                        nc.vector.tensor_copy(out=dst[:, mo:mo + mw],
                                              in_=pt[:, 0:2 * mw:2])
                feats = (z0s[0:C, 0:w], z1s[:, 0:w], z2s[:, 0:w])
                feats_x = (z0s[0:D_IN, 0:w], z1s[0:D_IN, 0:w],
                           z2s[0:D_IN, 0:w])
                psf = pspool.tile([BH, 512], F32, tag="ps", name="psf") \
                    if b == 0 else psf
                psu = pspool.tile([BH, 512], F32, tag="ps", name="psu") \
                    if b == 0 else psu
                psx = pspool.tile([BH, 512], F32, tag="ps", name="psx") \
                    if b == 0 else psx
                for k in range(3):
                    nc.tensor.matmul(psf[rows, 0:w], lhsT=wf_sb[k],
                                     rhs=feats[k], start=(k == 0),
                                     stop=(k == 2))
                for k in range(3):
                    nc.tensor.matmul(psu[rows, 0:w], lhsT=wu_sb[k],
                                     rhs=feats[k], start=(k == 0),
                                     stop=(k == 2))
                for k in range(3):
                    nc.tensor.matmul(psx[rows, 0:w], lhsT=wcx_sb[k],
                                     rhs=feats_x[k], start=(k == 0),
                                     stop=(k == 2))
            # gate nonlinearities + rh, full 128 partitions
            rst = sBpool.tile([BH, 512], F16, tag="rst", name="rst")
            nc.scalar.activation(rst[:, 0:w], psf[:, 0:w], SIG, bias=bf_sb[:, :])
            nc.scalar.activation(u_st[:, m0:m0 + w], psu[:, 0:w], SIG,
                                 bias=bu_sb[:, :])
            nc.gpsimd.tensor_mul(out=rh_st[:, m0:m0 + w], in0=rst[:, 0:w],
                                 in1=h_st[:, m0:m0 + w])
            nc.vector.tensor_copy(out=c_part[:, m0:m0 + w], in_=psx[:, 0:w])

        def post2b(g):
            # rhT for the candidate diffusion (node-major fp8); staggered a
            # second band behind so the sigma->rh round trip has completed
            ml = _mlist(g)
            for b in range(B_LOC):
                rows = slice(b * D_H, (b + 1) * D_H)
                for mi, (mo, mw) in enumerate(ml):
                    ch = g * 4 + mi
                    ptr = pspool.tile([CHUNK, D_H], F16, tag="ps", name="ptr")
                    nc.tensor.transpose(
                        ptr[:, :],
                        rh_st[rows, ch * CHUNK:(ch + 1) * CHUNK],
                        idm[rows, rows])
                    nc.vector.tensor_copy(
                        out=rhT3[:, ch, b * D_H:(b + 1) * D_H], in_=ptr[:, :])

        # phase 2 driver: dependent work staggered behind the sa matmuls so
        # the ACT/DVE/Pool round trips hide under PE work
        for g in range(NBAND):
            sa_band(g, z13, z23, S_Z2E)
            if g > 0:
                post2(g - 1)
            if g > 1:
                post2b(g - 2)
        post2(NBAND - 1)
        post2b(NBAND - 2)
        post2b(NBAND - 1)

        # ---- phase 3: zc1 = A rh (activations stationary, adj moving) ----
        def mm_sz(g, lhsT3, name):
            psc = pspool.tile([BH, 512], F32, tag="ps", name=name)
            w = BW[g]
            for jj in range(NPR):
                nc.tensor.matmul(
                    psc[:, 0:w],
                    lhsT=lhsT3[:, 2 * jj:2 * jj + 2, :],
                    rhs=at_sb[g][:, 2 * jj:2 * jj + 2, 0:w],
                    start=(jj == 0), stop=(jj == NPR - 1), perf_mode=DR)
            return psc

        def post3(g):
            for b in range(B_LOC):
                rows = slice(b * D_H, (b + 1) * D_H)
                for mi, (mo, mw) in enumerate(_mlist(g)):
                    ch = g * 4 + mi
                    ptc = pspool.tile([CHUNK, 2 * D_H], F8, tag="ps",
                                      name="ptc")
                    nc.tensor.transpose(
                        ptc[:, 0:2 * D_H:2],
                        zc1_bm[rows, ch * CHUNK:(ch + 1) * CHUNK],
                        idm8[rows, rows])
                    nc.vector.tensor_copy(
                        out=zc1T3[:, ch, b * D_H:(b + 1) * D_H],
                        in_=ptc[:, 0:2 * D_H:2])

        for g in range(NBAND):
            m0, w = BOFF[g], BW[g]
            psc = mm_sz(g, rhT3, "psc")
            nc.scalar.activation(zc1_bm[:, m0:m0 + w], psc[:, 0:w], COPY,
                                 scale=S_C1E)
            if g > 0:
                post3(g - 1)
        post3(NBAND - 1)

        # ---- phase 4: zc2 = A zc1, fused candidate conv + combine ----
        def cons4(g, psc2):
            m0, w = BOFF[g], BW[g]
            zc2s = sBpool.tile([BH, 512], F16, tag="zc2s", name="zc2s")
            nc.scalar.activation(zc2s[:, 0:w], psc2[:, 0:w], COPY,
                                 scale=S_C2E)
            zc1s = sBpool.tile([BH, 512], F16, tag="zc1s", name="zc1s")
            nc.vector.tensor_copy(out=zc1s[:, 0:w], in_=zc1_bm[:, m0:m0 + w])
            # batch-1 features need base partition 0: SBUF->SBUF DMA restage
            b1rh = sApool.tile([D_H, 512], F16, tag="b1rh", name="b1rh")
            nc.scalar.dma_start(out=b1rh[:, 0:w], in_=rh_st[D_H:BH, m0:m0 + w])
            b1c1 = sApool.tile([D_H, 512], F16, tag="b1c1", name="b1c1")
            nc.scalar.dma_start(out=b1c1[:, 0:w], in_=zc1s[D_H:BH, 0:w])
            b1c2 = sApool.tile([D_H, 512], F16, tag="b1c2", name="b1c2")
            nc.scalar.dma_start(out=b1c2[:, 0:w], in_=zc2s[D_H:BH, 0:w])
            psc3 = pspool.tile([BH, 512], F32, tag="ps", name="psc3")
            for b in range(B_LOC):
                rows = slice(b * D_H, (b + 1) * D_H)
                terms = ((rh_st[0:D_H, m0:m0 + w], zc1s[0:D_H, 0:w],
                          zc2s[0:D_H, 0:w]) if b == 0 else
                         (b1rh[:, 0:w], b1c1[:, 0:w], b1c2[:, 0:w]))
                for k in range(3):
                    nc.tensor.matmul(psc3[rows, 0:w], lhsT=wcrh_sb[k],
                                     rhs=terms[k], start=(k == 0),
                                     stop=(k == 2))
            tt = sCpool.tile([BH, 512], F16, tag="tt", name="tt")
            nc.vector.tensor_add(out=tt[:, 0:w], in0=psc3[:, 0:w],
                                 in1=c_part[:, m0:m0 + w])
            cst = sCpool.tile([BH, 512], F32, tag="cst", name="cst")
            nc.scalar.activation(cst[:, 0:w], tt[:, 0:w], TANH,
                                 bias=bc_sb[:, :])
            # combine on the otherwise-idle Pool engine; tt holds h-c
            nc.gpsimd.tensor_sub(out=tt[:, 0:w], in0=h_st[:, m0:m0 + w],
                                 in1=cst[:, 0:w])
            nc.gpsimd.tensor_mul(out=tt[:, 0:w], in0=u_st[:, m0:m0 + w],
                                 in1=tt[:, 0:w])
            nc.gpsimd.tensor_add(out=cst[:, 0:w], in0=cst[:, 0:w],
                                 in1=tt[:, 0:w])
            # out DMAs split across rings: trigger-instruction time on one
            # ring otherwise outlasts the compute tail
            nc.sync.dma_start(out=out_d[0][:, m0:m0 + w],
                              in_=cst[0:D_H, 0:w])
            nc.scalar.dma_start(out=out_d[1][:, m0:m0 + w],
                                in_=cst[D_H:BH, 0:w])

        psc2_prev = None
        for g in range(NBAND):
            psc2 = mm_sz(g, zc1T3, "psc2")
            if g > 0:
                cons4(g - 1, psc2_prev)
            psc2_prev = psc2
        cons4(NBAND - 1, psc2_prev)


# ---- host-side driver ----
_CACHED_NC = None
TRACE = False           # set True (e.g. from test.py) to capture an NTFF profile
TRACE_DIR = None
LAST_RESULTS = None     # BassKernelResults of the most recent kernel() call


def _host_prep(x, h, adj, Wf, bf, Wu, bu, Wc, bc):
    """Shard + cast + layout inputs for the 8 cores. Returns list of in_maps."""
    atp = np.zeros((NP, NN), dtype=np.float32)
    atp[:NN] = adj.T * 4096.0
    at8 = atp.astype(E4NP)                       # [4096, 4000]
    blocks = at8.reshape(NCH, CHUNK, NN)
    cols = [np.ascontiguousarray(
        blocks[:, :, BOFF[g]:BOFF[g] + BW[g]].transpose(1, 0, 2)
    ).reshape(CHUNK, NCH * BW[g]) for g in range(NBAND)]
    at_h = np.ascontiguousarray(np.concatenate(cols, axis=1))

    id16 = np.eye(CHUNK, dtype=np.float16)
    id8 = np.eye(CHUNK, dtype=E4NP)

    wsc = {"wf": (1.0, 1 / 32., 1 / 512.), "wu": (1.0, 1 / 32., 1 / 512.),
           "wcx": (1.0, 1 / 32., 1 / 512.), "wcrh": (1.0, 1 / 64., 1 / 512.)}

    def wsplit(W, key, lo, hi):
        return np.ascontiguousarray(np.stack(
            [(W[:, k * C + lo:k * C + hi].T * wsc[key][k]).astype(np.float16)
             for k in range(3)]))

    def bstack(v):
        return np.concatenate([v] * B_LOC).reshape(BH, 1).astype(np.float32)

    shared = {
        "at": at_h, "id16": id16, "id8": id8,
        "wf": wsplit(Wf, "wf", 0, C), "wu": wsplit(Wu, "wu", 0, C),
        "wcx": wsplit(Wc, "wcx", 0, D_IN), "wcrh": wsplit(Wc, "wcrh", D_IN, C),
        "bf": bstack(bf), "bu": bstack(bu), "bcb": bstack(bc),
    }
    in_maps = []
    for core in range(NCORES):
        bs = slice(core * B_LOC, (core + 1) * B_LOC)
        z = np.concatenate([x[bs], h[bs]], axis=1)       # [B_LOC, C, NN]
        znm = z.transpose(2, 0, 1)                       # [NN, B_LOC, C]
        ztp = np.zeros((NP, BC), dtype=np.float32)
        ztp[:NN] = znm.reshape(NN, BC)
        zt8 = np.ascontiguousarray(
            ztp.astype(E4NP).reshape(NCH, CHUNK, BC).transpose(1, 0, 2)
        ).reshape(CHUNK, NCH * BC)
        znp = np.zeros((NP, B_LOC, CHUNK), dtype=np.float16)
        znp[:NN, :, :C] = znm
        h_p = np.ascontiguousarray(
            h[bs].astype(np.float16).reshape(BH, NN))
        in_maps.append(dict(shared, zt=zt8, zn=znp, h=h_p))
    return in_maps


def kernel(**inputs):
    global _CACHED_NC, LAST_RESULTS
    inputs = {k: np.asarray(v) for k, v in inputs.items()}
    if _CACHED_NC is None:
        _CACHED_NC = build_program()
    in_maps = _host_prep(**inputs)
    kw = {}
    if TRACE:
        kw = dict(trace=True, tmpdir=TRACE_DIR)
    res = run_bass_kernel_spmd(_CACHED_NC, in_maps,
                               core_ids=list(range(NCORES)), **kw)
    LAST_RESULTS = res
    outs = [res.results[i]["out"] for i in range(NCORES)]
    return np.concatenate(outs, axis=0).astype(np.float32)


if __name__ == "__main__":
    rng = np.random.default_rng(0)
    ins = {
        "x": rng.standard_normal((B, D_IN, NN), dtype=np.float32),
        "h": rng.standard_normal((B, D_H, NN), dtype=np.float32),
        "adj": rng.random((NN, NN), dtype=np.float32) / NN,
        "Wf": rng.standard_normal((D_H, 3 * C), dtype=np.float32) * 0.05,
        "Wu": rng.standard_normal((D_H, 3 * C), dtype=np.float32) * 0.05,
        "Wc": rng.standard_normal((D_H, 3 * C), dtype=np.float32) * 0.05,
        "bf": rng.standard_normal(D_H).astype(np.float32) * 0.05,
        "bu": rng.standard_normal(D_H).astype(np.float32) * 0.05,
        "bc": rng.standard_normal(D_H).astype(np.float32) * 0.05,
    }
    out = kernel(**ins)
    print(out.shape, out.dtype)


# revision 30
# speedup vs baseline: 1.1844x; 1.0481x over previous
"""GCGRU cell (order-2 graph diffusion GRU) Trainium2 Bass kernel, v2.

Strategy: data-parallel over batch (B=16 -> 2 batches per core x 8 cores).
The dominant cost in v1 was streaming the 32MB fp16 adjacency from HBM four
times per core (DMA 99% busy). v2 keeps the whole adjacency RESIDENT in SBUF
as fp8 (x4096 pre-scale keeps the row-normalized values out of e4m3's
denormal range), loaded once (~16MB), and runs all four diffusion passes as
fp8 DoubleRow matmuls (2 packed contraction rows/cycle). Diffused features
are small contributors to the output (the graph averages 4000 nodes), so fp8
error lands ~1e-4 relative; order-k features carry power-of-2 scales folded
into the PSUM-evacuation copies and undone by host-side weight pre-scaling.

Layouts per core: activations node-major fp8 [128p x (chunk, col)] for
diffusion; gate/candidate convs run fp16 from per-band staging tiles
(PE transposes for diffused features, XBAR DMA-transpose from DRAM for the
raw [x;h] features). Gate/candidate nonlinearities on ACT, elementwise on
DVE, combine fused into the last diffusion's band loop.
"""

import numpy as np
import ml_dtypes

import concourse.bass as bass
from concourse import bacc
import concourse.mybir as mybir
import concourse.tile as tile
from concourse.bass_utils import run_bass_kernel_spmd

# problem constants
B, D_IN, D_H, NN = 16, 32, 64, 4000
NCORES = 8
B_LOC = B // NCORES          # batches per core
C = D_IN + D_H               # 96 channels into each gate conv
BC = B_LOC * C               # node-major column count (b-major: [b0 c96 | b1 c96])
BH = B_LOC * D_H             # stacked batch-hidden rows (128)
NP = 4096                    # contraction node dim padded to 32 chunks
CHUNK = 128
NCH = NP // CHUNK            # 32 contraction chunks
NPR = NCH // 2               # 16 DoubleRow chunk pairs
NBAND = 8                    # output-node bands: 7x512 + 416 (= 4000, no pad)
BW = [512] * 7 + [416]
BOFF = [512 * g for g in range(NBAND)]
AOFF = [NCH * 512 * g for g in range(NBAND)]   # at_d col offset per band

F8 = mybir.dt.float8e4
F16 = mybir.dt.float16
F32 = mybir.dt.float32
DR = mybir.MatmulPerfMode.DoubleRow
E4NP = ml_dtypes.float8_e4m3

# fp8 scale chain: adjacency carries x4096 (2^12).
#   z1T carries x32   -> evac scale 32/4096
#   z2T carries x512  -> evac scale 512/(4096*32)
#   zc1 carries x64   -> evac scale 64/4096
#   zc2 stage x512    -> evac scale 512/(4096*64)
S_Z1E, S_Z2E = 2.0 ** -7, 2.0 ** -8
S_C1E, S_C2E = 2.0 ** -6, 2.0 ** -9
# matching host-side weight descales: gate W1 /32, W2 /512; cand x-part
# W1 /32, W2 /512; cand rh-part W1 /64, W2 /512.


def _mlist(g):
    """(offset, width) of the 128-wide m-chunks inside band g."""
    w = BW[g]
    out = []
    mo = 0
    while mo < w:
        out.append((mo, min(CHUNK, w - mo)))
        mo += CHUNK
    return out


def build_program():
    nc = bacc.Bacc("TRN2", target_bir_lowering=False, debug=False)

    at_d = nc.dram_tensor("at", [CHUNK, NCH * NN], F8, kind="ExternalInput").ap()
    zt_d = nc.dram_tensor("zt", [CHUNK, NCH * BC], F8, kind="ExternalInput").ap()
    # node-major [x;h] fp16, padded to 128 cols/batch for XBAR dma transpose
    zn_d = nc.dram_tensor("zn", [NP, B_LOC, CHUNK], F16, kind="ExternalInput").ap()
    h_d = nc.dram_tensor("h", [BH, NN], F16, kind="ExternalInput").ap()
    wf_d = nc.dram_tensor("wf", [3, C, D_H], F16, kind="ExternalInput").ap()
    wu_d = nc.dram_tensor("wu", [3, C, D_H], F16, kind="ExternalInput").ap()
    wcx_d = nc.dram_tensor("wcx", [3, D_IN, D_H], F16, kind="ExternalInput").ap()
    wcrh_d = nc.dram_tensor("wcrh", [3, D_H, D_H], F16, kind="ExternalInput").ap()
    bf_d = nc.dram_tensor("bf", [BH, 1], F32, kind="ExternalInput").ap()
    bu_d = nc.dram_tensor("bu", [BH, 1], F32, kind="ExternalInput").ap()
    bc_d = nc.dram_tensor("bcb", [BH, 1], F32, kind="ExternalInput").ap()
    id16_d = nc.dram_tensor("id16", [CHUNK, CHUNK], F16, kind="ExternalInput").ap()
    id8_d = nc.dram_tensor("id8", [CHUNK, CHUNK], F8, kind="ExternalInput").ap()
    out_d = nc.dram_tensor("out", [B_LOC, D_H, NN], F32, kind="ExternalOutput").ap()

    with tile.TileContext(nc) as tc:
        _body(tc, locals())
    nc.compile()
    return nc


def _body(tc, aps):
    nc = tc.nc
    at_d, zt_d, zn_d, h_d = aps["at_d"], aps["zt_d"], aps["zn_d"], aps["h_d"]
    wf_d, wu_d, wcx_d, wcrh_d = (
        aps["wf_d"], aps["wu_d"], aps["wcx_d"], aps["wcrh_d"])
    bf_d, bu_d, bc_d = aps["bf_d"], aps["bu_d"], aps["bc_d"]
    id16_d, id8_d, out_d = aps["id16_d"], aps["id8_d"], aps["out_d"]

    SIG = mybir.ActivationFunctionType.Sigmoid
    TANH = mybir.ActivationFunctionType.Tanh
    COPY = mybir.ActivationFunctionType.Copy

    with (
        tc.tile_pool(name="const", bufs=1) as cpool,
        tc.tile_pool(name="amat", bufs=1) as apool,       # resident adjacency
        tc.tile_pool(name="nm8", bufs=2) as nmpool,       # rotating node-major fp8
        tc.tile_pool(name="perst", bufs=1) as ppool,
        tc.tile_pool(name="stageA", bufs=2) as sApool,    # conv feature stages
        tc.tile_pool(name="stageB", bufs=2) as sBpool,    # wide f16 stages
        tc.tile_pool(name="stageC", bufs=2) as sCpool,    # f32 combine stages
        tc.tile_pool(name="psum", bufs=8, space="PSUM") as pspool,
    ):
        # ---- persistent loads ----
        # DMA priority: phase 1 is gated on ztT + at0, so those go first on
        # separate rings; weights/h/idm are not needed until phase 2.
        # small loads first — they must not queue behind the ring-throttled
        # adjacency triggers
        idm = cpool.tile([CHUNK, CHUNK], F16, tag="idm")
        nc.sync.dma_start(out=idm[:], in_=id16_d[:])
        idm8 = cpool.tile([CHUNK, CHUNK], F8, tag="idm8")
        nc.sync.dma_start(out=idm8[:], in_=id8_d[:])
        bf_sb = cpool.tile([BH, 1], F32, tag="bf")
        nc.sync.dma_start(out=bf_sb[:], in_=bf_d[:])
        bu_sb = cpool.tile([BH, 1], F32, tag="bu")
        nc.sync.dma_start(out=bu_sb[:], in_=bu_d[:])
        bc_sb = cpool.tile([BH, 1], F32, tag="bc")
        nc.sync.dma_start(out=bc_sb[:], in_=bc_d[:])
        wf_sb = [cpool.tile([C, D_H], F16, tag=f"wf{k}", name=f"wf{k}")
                 for k in range(3)]
        wu_sb = [cpool.tile([C, D_H], F16, tag=f"wu{k}", name=f"wu{k}")
                 for k in range(3)]
        wcx_sb = [cpool.tile([D_IN, D_H], F16, tag=f"wcx{k}", name=f"wcx{k}")
                  for k in range(3)]
        wcrh_sb = [cpool.tile([D_H, D_H], F16, tag=f"wcrh{k}", name=f"wcrh{k}")
                   for k in range(3)]
        for k in range(3):
            nc.scalar.dma_start(out=wf_sb[k][:], in_=wf_d[k])
            nc.scalar.dma_start(out=wu_sb[k][:], in_=wu_d[k])
            nc.scalar.dma_start(out=wcx_sb[k][:], in_=wcx_d[k])
            nc.scalar.dma_start(out=wcrh_sb[k][:], in_=wcrh_d[k])

        ztT = nmpool.tile([CHUNK, NCH * BC], F8, tag="nm", name="ztT")
        nc.scalar.dma_start(out=ztT[:, :], in_=zt_d[:, :])
        # resident adjacency^T (x4096, fp8), one tile per output band.
        # Band 0 is latency-critical: split across both rings. Later bands
        # alternate whole-band per ring (trigger-instruction throughput is
        # the binding resource, not HBM bandwidth).
        at_sb = []
        at_t = []
        for g in range(NBAND):
            t = apool.tile([CHUNK, NCH * BW[g]], F8, tag=f"at{g}",
                           name=f"at{g}")
            at_t.append(t)
            at_sb.append(t[:, :].rearrange("p (j m) -> p j m", j=NCH))
        for g in range(4):
            half = (NCH // 2) * BW[g]
            nc.sync.dma_start(out=at_t[g][:, 0:half],
                              in_=at_d[:, AOFF[g]:AOFF[g] + half])
            nc.scalar.dma_start(
                out=at_t[g][:, half:NCH * BW[g]],
                in_=at_d[:, AOFF[g] + half:AOFF[g] + NCH * BW[g]])
        for g in range(4, NBAND):
            eng = nc.scalar if g % 2 == 1 else nc.sync
            eng.dma_start(out=at_t[g][:],
                          in_=at_d[:, AOFF[g]:AOFF[g] + NCH * BW[g]])

        # h is not needed until the phase-2 gate math (~60us in)
        h_st = ppool.tile([BH, NN], F16, tag="h_st")
        nc.scalar.dma_start(out=h_st[:], in_=h_d[:])

        u_st = ppool.tile([BH, NN], F16, tag="u_st")
        rh_st = ppool.tile([BH, NP], F16, tag="rh_st")
        nc.vector.memset(rh_st[:, NN:NP], 0.0)
        c_part = ppool.tile([BH, NN], F16, tag="c_part")
        rhT = ppool.tile([CHUNK, NCH * BH], F8, tag="rhT")
        zc1_bm = ppool.tile([BH, NP], F8, tag="zc1_bm")
        nc.vector.memset(zc1_bm[:, NN:NP], 0.0)
        zc1T = ppool.tile([CHUNK, NCH * BH], F8, tag="zc1T")

        zt3 = ztT[:, :].rearrange("p (j f) -> p j f", j=NCH)
        rhT3 = rhT[:, :].rearrange("p (j f) -> p j f", j=NCH)
        zc1T3 = zc1T[:, :].rearrange("p (j f) -> p j f", j=NCH)

        def sa_band(g, src3, dst3, evac_scale):
            """band g of dst = A @ src, node-major -> node-major."""
            ml = _mlist(g)
            pss = [pspool.tile([CHUNK, BC], F32, tag="ps", name=f"psd{mi}")
                   for mi in range(len(ml))]
            for jj in range(NPR):
                for mi, (mo, mw) in enumerate(ml):
                    nc.tensor.matmul(
                        pss[mi][0:mw, :],
                        lhsT=at_sb[g][:, 2 * jj:2 * jj + 2, mo:mo + mw],
                        rhs=src3[:, 2 * jj:2 * jj + 2, :],
                        start=(jj == 0), stop=(jj == NPR - 1), perf_mode=DR)
            for mi, (mo, mw) in enumerate(ml):
                # evac on DVE: the ACT queue holds the ring-throttled
                # adjacency DMA triggers early on and must not gate PSUM reuse
                nc.vector.tensor_scalar_mul(
                    out=dst3[0:mw, g * 4 + mi, :], in0=pss[mi][0:mw, :],
                    scalar1=evac_scale)

        # ---- phase 1: z1 = A z ----
        z1T = nmpool.tile([CHUNK, NCH * BC], F8, tag="nm", name="z1T")
        z13 = z1T[:, :].rearrange("p (j f) -> p j f", j=NCH)
        nc.vector.memset(z13[:, NCH - 1, :], 0.0)
        for g in range(NBAND):
            sa_band(g, zt3, z13, S_Z1E)

        # ---- phase 2: z2 = A z1, fused with gate convs, rh, rhT ----
        z2T = nmpool.tile([CHUNK, NCH * BC], F8, tag="nm", name="z2T")
        z23 = z2T[:, :].rearrange("p (j f) -> p j f", j=NCH)
        nc.vector.memset(z23[:, NCH - 1, :], 0.0)

        def post2(g):
            ml = _mlist(g)
            m0, w = BOFF[g], BW[g]
            for b in range(B_LOC):
                rows = slice(b * D_H, (b + 1) * D_H)
                # stage conv features (fp16, base partition 0)
                z0s = sBpool.tile([CHUNK, 512], F16, tag="z0s", name="z0s")
                nc.sync.dma_start_transpose(
                    out=z0s[:, 0:w], in_=zn_d[m0:m0 + w, b, :])
                z1s = sApool.tile([C, 512], F16, tag="z1s", name="z1s")
                z2s = sApool.tile([C, 512], F16, tag="z2s", name="z2s")
                for src3, dst in ((z13, z1s), (z23, z2s)):
                    for mi, (mo, mw) in enumerate(ml):
                        # fp8 PE transpose writes PSUM at element step 2
                        pt = pspool.tile([C, 2 * CHUNK], F8, tag="ps",
                                         name="pt")
                        nc.tensor.transpose(
                            pt[:, 0:2 * CHUNK:2],
                            src3[:, g * 4 + mi, b * C:(b + 1) * C],
                            idm8[:, :])
                        nc.vector.tensor_copy(out=dst[:, mo:mo + mw],
                                              in_=pt[:, 0:2 * mw:2])
                feats = (z0s[0:C, 0:w], z1s[:, 0:w], z2s[:, 0:w])
                feats_x = (z0s[0:D_IN, 0:w], z1s[0:D_IN, 0:w],
                           z2s[0:D_IN, 0:w])
                psf = pspool.tile([BH, 512], F32, tag="ps", name="psf") \
                    if b == 0 else psf
                psu = pspool.tile([BH, 512], F32, tag="ps", name="psu") \
                    if b == 0 else psu
                psx = pspool.tile([BH, 512], F32, tag="ps", name="psx") \
                    if b == 0 else psx
                for k in range(3):
                    nc.tensor.matmul(psf[rows, 0:w], lhsT=wf_sb[k],
                                     rhs=feats[k], start=(k == 0),
                                     stop=(k == 2))
                for k in range(3):
                    nc.tensor.matmul(psu[rows, 0:w], lhsT=wu_sb[k],
                                     rhs=feats[k], start=(k == 0),
                                     stop=(k == 2))
                for k in range(3):
                    nc.tensor.matmul(psx[rows, 0:w], lhsT=wcx_sb[k],
                                     rhs=feats_x[k], start=(k == 0),
                                     stop=(k == 2))
            # gate nonlinearities + rh, full 128 partitions
            rst = sBpool.tile([BH, 512], F16, tag="rst", name="rst")
            nc.scalar.activation(rst[:, 0:w], psf[:, 0:w], SIG, bias=bf_sb[:, :])
            nc.scalar.activation(u_st[:, m0:m0 + w], psu[:, 0:w], SIG,
                                 bias=bu_sb[:, :])
            nc.gpsimd.tensor_mul(out=rh_st[:, m0:m0 + w], in0=rst[:, 0:w],
                                 in1=h_st[:, m0:m0 + w])
            nc.vector.tensor_copy(out=c_part[:, m0:m0 + w], in_=psx[:, 0:w])

        def post2b(g):
            # rhT for the candidate diffusion (node-major fp8); staggered a
            # second band behind so the sigma->rh round trip has completed.
            # rh_st rows are batch-stacked, so one full-128 transpose per
            # chunk yields the [b*64+r] column layout directly.
            for mi, (mo, mw) in enumerate(_mlist(g)):
                ch = g * 4 + mi
                ptr = pspool.tile([CHUNK, CHUNK], F16, tag="ps", name="ptr")
                nc.tensor.transpose(
                    ptr[:, :], rh_st[:, ch * CHUNK:(ch + 1) * CHUNK],
                    idm[:, :])
                nc.vector.tensor_copy(out=rhT3[:, ch, :], in_=ptr[:, :])

        # phase 2 driver: dependent work staggered behind the sa matmuls so
        # the ACT/DVE/Pool round trips hide under PE work
        for g in range(NBAND):
            sa_band(g, z13, z23, S_Z2E)
            if g > 0:
                post2(g - 1)
            if g > 1:
                post2b(g - 2)
        post2(NBAND - 1)
        post2b(NBAND - 2)
        post2b(NBAND - 1)

        # ---- phase 3: zc1 = A rh (activations stationary, adj moving) ----
        def mm_sz(g, lhsT3, name):
            psc = pspool.tile([BH, 512], F32, tag="ps", name=name)
            w = BW[g]
            for jj in range(NPR):
                nc.tensor.matmul(
                    psc[:, 0:w],
                    lhsT=lhsT3[:, 2 * jj:2 * jj + 2, :],
                    rhs=at_sb[g][:, 2 * jj:2 * jj + 2, 0:w],
                    start=(jj == 0), stop=(jj == NPR - 1), perf_mode=DR)
            return psc

        def post3(g):
            # zc1_bm rows are batch-stacked: one full-128 fp8 transpose per
            # chunk (PSUM element step 2) gives the zc1T column layout
            for mi, (mo, mw) in enumerate(_mlist(g)):
                ch = g * 4 + mi
                ptc = pspool.tile([CHUNK, 2 * CHUNK], F8, tag="ps",
                                  name="ptc")
                nc.tensor.transpose(
                    ptc[:, 0:2 * CHUNK:2],
                    zc1_bm[:, ch * CHUNK:(ch + 1) * CHUNK],
                    idm8[:, :])
                nc.vector.tensor_copy(
                    out=zc1T3[:, ch, :], in_=ptc[:, 0:2 * CHUNK:2])

        for g in range(NBAND):
            m0, w = BOFF[g], BW[g]
            psc = mm_sz(g, rhT3, "psc")
            nc.scalar.activation(zc1_bm[:, m0:m0 + w], psc[:, 0:w], COPY,
                                 scale=S_C1E)
            if g > 0:
                post3(g - 1)
        post3(NBAND - 1)

        # ---- phase 4: zc2 = A zc1, fused candidate conv + combine ----
        def cons4(g, psc2):
            m0, w = BOFF[g], BW[g]
            zc2s = sBpool.tile([BH, 512], F16, tag="zc2s", name="zc2s")
            nc.scalar.activation(zc2s[:, 0:w], psc2[:, 0:w], COPY,
                                 scale=S_C2E)
            zc1s = sBpool.tile([BH, 512], F16, tag="zc1s", name="zc1s")
            nc.vector.tensor_copy(out=zc1s[:, 0:w], in_=zc1_bm[:, m0:m0 + w])
            # batch-1 features need base partition 0: SBUF->SBUF DMA restage
            b1rh = sApool.tile([D_H, 512], F16, tag="b1rh", name="b1rh")
            nc.scalar.dma_start(out=b1rh[:, 0:w], in_=rh_st[D_H:BH, m0:m0 + w])
            b1c1 = sApool.tile([D_H, 512], F16, tag="b1c1", name="b1c1")
            nc.scalar.dma_start(out=b1c1[:, 0:w], in_=zc1s[D_H:BH, 0:w])
            b1c2 = sApool.tile([D_H, 512], F16, tag="b1c2", name="b1c2")
            nc.scalar.dma_start(out=b1c2[:, 0:w], in_=zc2s[D_H:BH, 0:w])
            psc3 = pspool.tile([BH, 512], F32, tag="ps", name="psc3")
            for b in range(B_LOC):
                rows = slice(b * D_H, (b + 1) * D_H)
                terms = ((rh_st[0:D_H, m0:m0 + w], zc1s[0:D_H, 0:w],
                          zc2s[0:D_H, 0:w]) if b == 0 else
                         (b1rh[:, 0:w], b1c1[:, 0:w], b1c2[:, 0:w]))
                for k in range(3):
                    nc.tensor.matmul(psc3[rows, 0:w], lhsT=wcrh_sb[k],
                                     rhs=terms[k], start=(k == 0),
                                     stop=(k == 2))
            tt = sCpool.tile([BH, 512], F16, tag="tt", name="tt")
            nc.vector.tensor_add(out=tt[:, 0:w], in0=psc3[:, 0:w],
                                 in1=c_part[:, m0:m0 + w])
            cst = sCpool.tile([BH, 512], F32, tag="cst", name="cst")
            nc.scalar.activation(cst[:, 0:w], tt[:, 0:w], TANH,
                                 bias=bc_sb[:, :])
            # combine on the otherwise-idle Pool engine; tt holds h-c
            nc.gpsimd.tensor_sub(out=tt[:, 0:w], in0=h_st[:, m0:m0 + w],
                                 in1=cst[:, 0:w])
            nc.gpsimd.tensor_mul(out=tt[:, 0:w], in0=u_st[:, m0:m0 + w],
                                 in1=tt[:, 0:w])
            nc.gpsimd.tensor_add(out=cst[:, 0:w], in0=cst[:, 0:w],
                                 in1=tt[:, 0:w])
            # out DMAs split across rings: trigger-instruction time on one
            # ring otherwise outlasts the compute tail
            nc.sync.dma_start(out=out_d[0][:, m0:m0 + w],
                              in_=cst[0:D_H, 0:w])
            nc.scalar.dma_start(out=out_d[1][:, m0:m0 + w],
                                in_=cst[D_H:BH, 0:w])

        psc2_prev = None
        for g in range(NBAND):
            psc2 = mm_sz(g, zc1T3, "psc2")
            if g > 0:
                cons4(g - 1, psc2_prev)
            psc2_prev = psc2
        cons4(NBAND - 1, psc2_prev)


# ---- host-side driver ----
_CACHED_NC = None
TRACE = False           # set True (e.g. from test.py) to capture an NTFF profile
TRACE_DIR = None
LAST_RESULTS = None     # BassKernelResults of the most recent kernel() call


def _host_prep(x, h, adj, Wf, bf, Wu, bu, Wc, bc):
    """Shard + cast + layout inputs for the 8 cores. Returns list of in_maps."""
    atp = np.zeros((NP, NN), dtype=np.float32)
    atp[:NN] = adj.T * 4096.0
    at8 = atp.astype(E4NP)                       # [4096, 4000]
    blocks = at8.reshape(NCH, CHUNK, NN)
    cols = [np.ascontiguousarray(
        blocks[:, :, BOFF[g]:BOFF[g] + BW[g]].transpose(1, 0, 2)
    ).reshape(CHUNK, NCH * BW[g]) for g in range(NBAND)]
    at_h = np.ascontiguousarray(np.concatenate(cols, axis=1))

    id16 = np.eye(CHUNK, dtype=np.float16)
    id8 = np.eye(CHUNK, dtype=E4NP)

    wsc = {"wf": (1.0, 1 / 32., 1 / 512.), "wu": (1.0, 1 / 32., 1 / 512.),
           "wcx": (1.0, 1 / 32., 1 / 512.), "wcrh": (1.0, 1 / 64., 1 / 512.)}

    def wsplit(W, key, lo, hi):
        return np.ascontiguousarray(np.stack(
            [(W[:, k * C + lo:k * C + hi].T * wsc[key][k]).astype(np.float16)
             for k in range(3)]))

    def bstack(v):
        return np.concatenate([v] * B_LOC).reshape(BH, 1).astype(np.float32)

    shared = {
        "at": at_h, "id16": id16, "id8": id8,
        "wf": wsplit(Wf, "wf", 0, C), "wu": wsplit(Wu, "wu", 0, C),
        "wcx": wsplit(Wc, "wcx", 0, D_IN), "wcrh": wsplit(Wc, "wcrh", D_IN, C),
        "bf": bstack(bf), "bu": bstack(bu), "bcb": bstack(bc),
    }
    in_maps = []
    for core in range(NCORES):
        bs = slice(core * B_LOC, (core + 1) * B_LOC)
        z = np.concatenate([x[bs], h[bs]], axis=1)       # [B_LOC, C, NN]
        znm = z.transpose(2, 0, 1)                       # [NN, B_LOC, C]
        ztp = np.zeros((NP, BC), dtype=np.float32)
        ztp[:NN] = znm.reshape(NN, BC)
        zt8 = np.ascontiguousarray(
            ztp.astype(E4NP).reshape(NCH, CHUNK, BC).transpose(1, 0, 2)
        ).reshape(CHUNK, NCH * BC)
        znp = np.zeros((NP, B_LOC, CHUNK), dtype=np.float16)
        znp[:NN, :, :C] = znm
        h_p = np.ascontiguousarray(
            h[bs].astype(np.float16).reshape(BH, NN))
        in_maps.append(dict(shared, zt=zt8, zn=znp, h=h_p))
    return in_maps


def kernel(**inputs):
    global _CACHED_NC, LAST_RESULTS
    inputs = {k: np.asarray(v) for k, v in inputs.items()}
    if _CACHED_NC is None:
        _CACHED_NC = build_program()
    in_maps = _host_prep(**inputs)
    kw = {}
    if TRACE:
        kw = dict(trace=True, tmpdir=TRACE_DIR)
    res = run_bass_kernel_spmd(_CACHED_NC, in_maps,
                               core_ids=list(range(NCORES)), **kw)
    LAST_RESULTS = res
    outs = [res.results[i]["out"] for i in range(NCORES)]
    return np.concatenate(outs, axis=0).astype(np.float32)


if __name__ == "__main__":
    rng = np.random.default_rng(0)
    ins = {
        "x": rng.standard_normal((B, D_IN, NN), dtype=np.float32),
        "h": rng.standard_normal((B, D_H, NN), dtype=np.float32),
        "adj": rng.random((NN, NN), dtype=np.float32) / NN,
        "Wf": rng.standard_normal((D_H, 3 * C), dtype=np.float32) * 0.05,
        "Wu": rng.standard_normal((D_H, 3 * C), dtype=np.float32) * 0.05,
        "Wc": rng.standard_normal((D_H, 3 * C), dtype=np.float32) * 0.05,
        "bf": rng.standard_normal(D_H).astype(np.float32) * 0.05,
        "bu": rng.standard_normal(D_H).astype(np.float32) * 0.05,
        "bc": rng.standard_normal(D_H).astype(np.float32) * 0.05,
    }
    out = kernel(**ins)
    print(out.shape, out.dtype)


# revision 36
# speedup vs baseline: 1.2566x; 1.0610x over previous
"""GCGRU cell (order-2 graph diffusion GRU) Trainium2 Bass kernel, v2.

Strategy: data-parallel over batch (B=16 -> 2 batches per core x 8 cores).
The dominant cost in v1 was streaming the 32MB fp16 adjacency from HBM four
times per core (DMA 99% busy). v2 keeps the whole adjacency RESIDENT in SBUF
as fp8 (x4096 pre-scale keeps the row-normalized values out of e4m3's
denormal range), loaded once (~16MB), and runs all four diffusion passes as
fp8 DoubleRow matmuls (2 packed contraction rows/cycle). Diffused features
are small contributors to the output (the graph averages 4000 nodes), so fp8
error lands ~1e-4 relative; order-k features carry power-of-2 scales folded
into the PSUM-evacuation copies and undone by host-side weight pre-scaling.

Layouts per core: activations node-major fp8 [128p x (chunk, col)] for
diffusion; gate/candidate convs run fp16 from per-band staging tiles
(PE transposes for diffused features, XBAR DMA-transpose from DRAM for the
raw [x;h] features). Gate/candidate nonlinearities on ACT, elementwise on
DVE, combine fused into the last diffusion's band loop.
"""

import numpy as np
import ml_dtypes

import concourse.bass as bass
from concourse import bacc
import concourse.mybir as mybir
import concourse.tile as tile
from concourse.bass_utils import run_bass_kernel_spmd

# problem constants
B, D_IN, D_H, NN = 16, 32, 64, 4000
NCORES = 8
B_LOC = B // NCORES          # batches per core
C = D_IN + D_H               # 96 channels into each gate conv
BC = B_LOC * C               # node-major column count (b-major: [b0 c96 | b1 c96])
BH = B_LOC * D_H             # stacked batch-hidden rows (128)
NP = 4096                    # contraction node dim padded to 32 chunks
CHUNK = 128
NCH = NP // CHUNK            # 32 contraction chunks
NPR = NCH // 2               # 16 DoubleRow chunk pairs
NBAND = 8                    # output-node bands: 7x512 + 416 (= 4000, no pad)
BW = [512] * 7 + [416]
BOFF = [512 * g for g in range(NBAND)]
AOFF = [NCH * 512 * g for g in range(NBAND)]   # at_d col offset per band

F8 = mybir.dt.float8e4
F16 = mybir.dt.float16
F32 = mybir.dt.float32
DR = mybir.MatmulPerfMode.DoubleRow
E4NP = ml_dtypes.float8_e4m3

# fp8 scale chain: adjacency carries x4096 (2^12).
#   z1T carries x32   -> evac scale 32/4096
#   z2T carries x512  -> evac scale 512/(4096*32)
#   zc1 carries x64   -> evac scale 64/4096
#   zc2 stage x512    -> evac scale 512/(4096*64)
S_Z1E, S_Z2E = 2.0 ** -7, 2.0 ** -8
S_C1E, S_C2E = 2.0 ** -6, 2.0 ** -9
# matching host-side weight descales: gate W1 /32, W2 /512; cand x-part
# W1 /32, W2 /512; cand rh-part W1 /64, W2 /512.


def _mlist(g):
    """(offset, width) of the 128-wide m-chunks inside band g."""
    w = BW[g]
    out = []
    mo = 0
    while mo < w:
        out.append((mo, min(CHUNK, w - mo)))
        mo += CHUNK
    return out


def build_program():
    nc = bacc.Bacc("TRN2", target_bir_lowering=False, debug=False)

    at_d = nc.dram_tensor("at", [CHUNK, NCH * NN], F8, kind="ExternalInput").ap()
    zt_d = nc.dram_tensor("zt", [CHUNK, NCH * BC], F8, kind="ExternalInput").ap()
    # node-major [x;h] fp16, padded to 128 cols/batch for XBAR dma transpose
    zn_d = nc.dram_tensor("zn", [NP, B_LOC, CHUNK], F16, kind="ExternalInput").ap()
    h_d = nc.dram_tensor("h", [BH, NN], F16, kind="ExternalInput").ap()
    # all conv weights packed in one tensor: [wf0..2 | wu0..2 | wcx | wcrh]
    wall_d = nc.dram_tensor("wall", [C, 12 * D_H], F16,
                            kind="ExternalInput").ap()
    b3_d = nc.dram_tensor("b3", [BH, 3], F32, kind="ExternalInput").ap()
    id16_d = nc.dram_tensor("id16", [CHUNK, CHUNK], F16, kind="ExternalInput").ap()
    id8_d = nc.dram_tensor("id8", [CHUNK, CHUNK], F8, kind="ExternalInput").ap()
    out_d = nc.dram_tensor("out", [B_LOC, D_H, NN], F32, kind="ExternalOutput").ap()

    with tile.TileContext(nc) as tc:
        _body(tc, locals())
    nc.compile()
    return nc


def _body(tc, aps):
    nc = tc.nc
    at_d, zt_d, zn_d, h_d = aps["at_d"], aps["zt_d"], aps["zn_d"], aps["h_d"]
    wall_d, b3_d = aps["wall_d"], aps["b3_d"]
    id16_d, id8_d, out_d = aps["id16_d"], aps["id8_d"], aps["out_d"]

    SIG = mybir.ActivationFunctionType.Sigmoid
    TANH = mybir.ActivationFunctionType.Tanh
    COPY = mybir.ActivationFunctionType.Copy

    with (
        tc.tile_pool(name="const", bufs=1) as cpool,
        tc.tile_pool(name="amat", bufs=1) as apool,       # resident adjacency
        tc.tile_pool(name="nm8", bufs=2) as nmpool,       # rotating node-major fp8
        tc.tile_pool(name="perst", bufs=1) as ppool,
        tc.tile_pool(name="stageA", bufs=2) as sApool,    # conv feature stages
        tc.tile_pool(name="stageB", bufs=2) as sBpool,    # wide f16 stages
        tc.tile_pool(name="stageC", bufs=2) as sCpool,    # f32 combine stages
        tc.tile_pool(name="psum", bufs=8, space="PSUM") as pspool,
    ):
        # ---- persistent loads ----
        # DMA priority: phase 1 is gated on ztT + at0, so those go first on
        # separate rings; weights/h/idm are not needed until phase 2.
        # ring budget note: every DRAM->SBUF DMA costs one descriptor per
        # partition and ~0.6us of engine time per 16-descriptor trigger, so
        # the latency-critical adjacency goes first and everything small is
        # merged or deferred.
        ztT = nmpool.tile([CHUNK, NCH * BC], F8, tag="nm", name="ztT")
        nc.scalar.dma_start(out=ztT[:, :], in_=zt_d[:, :])
        # resident adjacency^T (x4096, fp8), one tile per output band.
        # Band 0 is latency-critical: split across both rings. Later bands
        # alternate whole-band per ring (trigger-instruction throughput is
        # the binding resource, not HBM bandwidth).
        at_sb = []
        at_t = []
        for g in range(NBAND):
            t = apool.tile([CHUNK, NCH * BW[g]], F8, tag=f"at{g}",
                           name=f"at{g}")
            at_t.append(t)
            at_sb.append(t[:, :].rearrange("p (j m) -> p j m", j=NCH))
        for g in range(4):
            half = (NCH // 2) * BW[g]
            nc.sync.dma_start(out=at_t[g][:, 0:half],
                              in_=at_d[:, AOFF[g]:AOFF[g] + half])
            nc.scalar.dma_start(
                out=at_t[g][:, half:NCH * BW[g]],
                in_=at_d[:, AOFF[g] + half:AOFF[g] + NCH * BW[g]])
        for g in range(4, NBAND):
            eng = nc.scalar if g % 2 == 1 else nc.sync
            eng.dma_start(out=at_t[g][:],
                          in_=at_d[:, AOFF[g]:AOFF[g] + NCH * BW[g]])

        # constants / weights / h: not needed until phase 2 (~60us in)
        idm = cpool.tile([CHUNK, CHUNK], F16, tag="idm")
        nc.sync.dma_start(out=idm[:], in_=id16_d[:])
        idm8 = cpool.tile([CHUNK, CHUNK], F8, tag="idm8")
        nc.sync.dma_start(out=idm8[:], in_=id8_d[:])
        b3_sb = cpool.tile([BH, 3], F32, tag="b3")
        nc.sync.dma_start(out=b3_sb[:], in_=b3_d[:])
        bf_sb, bu_sb, bc_sb = (b3_sb[:, 0:1], b3_sb[:, 1:2], b3_sb[:, 2:3])
        wall = cpool.tile([C, 12 * D_H], F16, tag="wall")
        nc.scalar.dma_start(out=wall[:], in_=wall_d[:])
        wf_sb = [wall[0:C, k * D_H:(k + 1) * D_H] for k in range(3)]
        wu_sb = [wall[0:C, (3 + k) * D_H:(4 + k) * D_H] for k in range(3)]
        wcx_sb = [wall[0:D_IN, (6 + k) * D_H:(7 + k) * D_H] for k in range(3)]
        wcrh_sb = [wall[0:D_H, (9 + k) * D_H:(10 + k) * D_H] for k in range(3)]
        h_st = ppool.tile([BH, NN], F16, tag="h_st")
        nc.scalar.dma_start(out=h_st[:], in_=h_d[:])

        u_st = ppool.tile([BH, NN], F16, tag="u_st")
        rh_st = ppool.tile([BH, NP], F16, tag="rh_st")
        nc.vector.memset(rh_st[:, NN:NP], 0.0)
        c_part = ppool.tile([BH, NN], F16, tag="c_part")
        rhT = ppool.tile([CHUNK, NCH * BH], F8, tag="rhT")
        zc1_bm = ppool.tile([BH, NP], F8, tag="zc1_bm")
        nc.vector.memset(zc1_bm[:, NN:NP], 0.0)
        zc1T = ppool.tile([CHUNK, NCH * BH], F8, tag="zc1T")

        zt3 = ztT[:, :].rearrange("p (j f) -> p j f", j=NCH)
        rhT3 = rhT[:, :].rearrange("p (j f) -> p j f", j=NCH)
        zc1T3 = zc1T[:, :].rearrange("p (j f) -> p j f", j=NCH)

        def sa_band(g, src3, dst3, evac_scale):
            """band g of dst = A @ src, node-major -> node-major."""
            ml = _mlist(g)
            pss = [pspool.tile([CHUNK, BC], F32, tag="ps", name=f"psd{mi}")
                   for mi in range(len(ml))]
            for jj in range(NPR):
                for mi, (mo, mw) in enumerate(ml):
                    nc.tensor.matmul(
                        pss[mi][0:mw, :],
                        lhsT=at_sb[g][:, 2 * jj:2 * jj + 2, mo:mo + mw],
                        rhs=src3[:, 2 * jj:2 * jj + 2, :],
                        start=(jj == 0), stop=(jj == NPR - 1), perf_mode=DR)
            for mi, (mo, mw) in enumerate(ml):
                # evac on DVE: the ACT queue holds the ring-throttled
                # adjacency DMA triggers early on and must not gate PSUM reuse
                nc.vector.tensor_scalar_mul(
                    out=dst3[0:mw, g * 4 + mi, :], in0=pss[mi][0:mw, :],
                    scalar1=evac_scale)

        # ---- phase 1: z1 = A z ----
        z1T = nmpool.tile([CHUNK, NCH * BC], F8, tag="nm", name="z1T")
        z13 = z1T[:, :].rearrange("p (j f) -> p j f", j=NCH)
        nc.vector.memset(z13[:, NCH - 1, :], 0.0)
        for g in range(NBAND):
            sa_band(g, zt3, z13, S_Z1E)

        # ---- phase 2: z2 = A z1, fused with gate convs, rh, rhT ----
        z2T = nmpool.tile([CHUNK, NCH * BC], F8, tag="nm", name="z2T")
        z23 = z2T[:, :].rearrange("p (j f) -> p j f", j=NCH)
        nc.vector.memset(z23[:, NCH - 1, :], 0.0)

        def post2(g):
            ml = _mlist(g)
            m0, w = BOFF[g], BW[g]
            for b in range(B_LOC):
                rows = slice(b * D_H, (b + 1) * D_H)
                # stage conv features (fp16, base partition 0)
                z0s = sBpool.tile([CHUNK, 512], F16, tag="z0s", name="z0s")
                nc.sync.dma_start_transpose(
                    out=z0s[:, 0:w], in_=zn_d[m0:m0 + w, b, :])
                z1s = sApool.tile([C, 512], F16, tag="z1s", name="z1s")
                z2s = sApool.tile([C, 512], F16, tag="z2s", name="z2s")
                for src3, dst in ((z13, z1s), (z23, z2s)):
                    for mi, (mo, mw) in enumerate(ml):
                        # fp8 PE transpose writes PSUM at element step 2
                        pt = pspool.tile([C, 2 * CHUNK], F8, tag="ps",
                                         name="pt")
                        nc.tensor.transpose(
                            pt[:, 0:2 * CHUNK:2],
                            src3[:, g * 4 + mi, b * C:(b + 1) * C],
                            idm8[:, :])
                        nc.vector.tensor_copy(out=dst[:, mo:mo + mw],
                                              in_=pt[:, 0:2 * mw:2])
                feats = (z0s[0:C, 0:w], z1s[:, 0:w], z2s[:, 0:w])
                feats_x = (z0s[0:D_IN, 0:w], z1s[0:D_IN, 0:w],
                           z2s[0:D_IN, 0:w])
                psf = pspool.tile([BH, 512], F32, tag="ps", name="psf") \
                    if b == 0 else psf
                psu = pspool.tile([BH, 512], F32, tag="ps", name="psu") \
                    if b == 0 else psu
                psx = pspool.tile([BH, 512], F32, tag="ps", name="psx") \
                    if b == 0 else psx
                for k in range(3):
                    nc.tensor.matmul(psf[rows, 0:w], lhsT=wf_sb[k],
                                     rhs=feats[k], start=(k == 0),
                                     stop=(k == 2))
                for k in range(3):
                    nc.tensor.matmul(psu[rows, 0:w], lhsT=wu_sb[k],
                                     rhs=feats[k], start=(k == 0),
                                     stop=(k == 2))
                for k in range(3):
                    nc.tensor.matmul(psx[rows, 0:w], lhsT=wcx_sb[k],
                                     rhs=feats_x[k], start=(k == 0),
                                     stop=(k == 2))
            # gate nonlinearities + rh, full 128 partitions
            rst = sBpool.tile([BH, 512], F16, tag="rst", name="rst")
            nc.scalar.activation(rst[:, 0:w], psf[:, 0:w], SIG, bias=bf_sb)
            nc.scalar.activation(u_st[:, m0:m0 + w], psu[:, 0:w], SIG,
                                 bias=bu_sb)
            nc.gpsimd.tensor_mul(out=rh_st[:, m0:m0 + w], in0=rst[:, 0:w],
                                 in1=h_st[:, m0:m0 + w])
            nc.vector.tensor_copy(out=c_part[:, m0:m0 + w], in_=psx[:, 0:w])

        def post2b(g):
            # rhT for the candidate diffusion (node-major fp8); staggered a
            # second band behind so the sigma->rh round trip has completed.
            # rh_st rows are batch-stacked, so one full-128 transpose per
            # chunk yields the [b*64+r] column layout directly.
            for mi, (mo, mw) in enumerate(_mlist(g)):
                ch = g * 4 + mi
                ptr = pspool.tile([CHUNK, CHUNK], F16, tag="ps", name="ptr")
                nc.tensor.transpose(
                    ptr[:, :], rh_st[:, ch * CHUNK:(ch + 1) * CHUNK],
                    idm[:, :])
                nc.vector.tensor_copy(out=rhT3[:, ch, :], in_=ptr[:, :])

        # phase 2 driver: dependent work staggered behind the sa matmuls so
        # the ACT/DVE/Pool round trips hide under PE work
        for g in range(NBAND):
            sa_band(g, z13, z23, S_Z2E)
            if g > 0:
                post2(g - 1)
            if g > 1:
                post2b(g - 2)
        post2(NBAND - 1)
        post2b(NBAND - 2)
        post2b(NBAND - 1)

        # ---- phase 3: zc1 = A rh (activations stationary, adj moving) ----
        def mm_sz(g, lhsT3, name):
            psc = pspool.tile([BH, 512], F32, tag="ps", name=name)
            w = BW[g]
            for jj in range(NPR):
                nc.tensor.matmul(
                    psc[:, 0:w],
                    lhsT=lhsT3[:, 2 * jj:2 * jj + 2, :],
                    rhs=at_sb[g][:, 2 * jj:2 * jj + 2, 0:w],
                    start=(jj == 0), stop=(jj == NPR - 1), perf_mode=DR)
            return psc

        def post3(g):
            # zc1_bm rows are batch-stacked: one full-128 fp8 transpose per
            # chunk (PSUM element step 2) gives the zc1T column layout
            for mi, (mo, mw) in enumerate(_mlist(g)):
                ch = g * 4 + mi
                ptc = pspool.tile([CHUNK, 2 * CHUNK], F8, tag="ps",
                                  name="ptc")
                nc.tensor.transpose(
                    ptc[:, 0:2 * CHUNK:2],
                    zc1_bm[:, ch * CHUNK:(ch + 1) * CHUNK],
                    idm8[:, :])
                nc.vector.tensor_copy(
                    out=zc1T3[:, ch, :], in_=ptc[:, 0:2 * CHUNK:2])

        for g in range(NBAND):
            m0, w = BOFF[g], BW[g]
            psc = mm_sz(g, rhT3, "psc")
            nc.scalar.activation(zc1_bm[:, m0:m0 + w], psc[:, 0:w], COPY,
                                 scale=S_C1E)
            if g > 0:
                post3(g - 1)
        post3(NBAND - 1)

        # ---- phase 4: zc2 = A zc1, fused candidate conv + combine ----
        def cons4(g, psc2):
            m0, w = BOFF[g], BW[g]
            zc2s = sBpool.tile([BH, 512], F16, tag="zc2s", name="zc2s")
            nc.scalar.activation(zc2s[:, 0:w], psc2[:, 0:w], COPY,
                                 scale=S_C2E)
            zc1s = sBpool.tile([BH, 512], F16, tag="zc1s", name="zc1s")
            nc.vector.tensor_copy(out=zc1s[:, 0:w], in_=zc1_bm[:, m0:m0 + w])
            # batch-1 features need base partition 0: SBUF->SBUF DMA restage
            b1rh = sApool.tile([D_H, 512], F16, tag="b1rh", name="b1rh")
            nc.scalar.dma_start(out=b1rh[:, 0:w], in_=rh_st[D_H:BH, m0:m0 + w])
            b1c1 = sApool.tile([D_H, 512], F16, tag="b1c1", name="b1c1")
            nc.scalar.dma_start(out=b1c1[:, 0:w], in_=zc1s[D_H:BH, 0:w])
            b1c2 = sApool.tile([D_H, 512], F16, tag="b1c2", name="b1c2")
            nc.scalar.dma_start(out=b1c2[:, 0:w], in_=zc2s[D_H:BH, 0:w])
            psc3 = pspool.tile([BH, 512], F32, tag="ps", name="psc3")
            for b in range(B_LOC):
                rows = slice(b * D_H, (b + 1) * D_H)
                terms = ((rh_st[0:D_H, m0:m0 + w], zc1s[0:D_H, 0:w],
                          zc2s[0:D_H, 0:w]) if b == 0 else
                         (b1rh[:, 0:w], b1c1[:, 0:w], b1c2[:, 0:w]))
                for k in range(3):
                    nc.tensor.matmul(psc3[rows, 0:w], lhsT=wcrh_sb[k],
                                     rhs=terms[k], start=(k == 0),
                                     stop=(k == 2))
            tt = sCpool.tile([BH, 512], F16, tag="tt", name="tt")
            nc.vector.tensor_add(out=tt[:, 0:w], in0=psc3[:, 0:w],
                                 in1=c_part[:, m0:m0 + w])
            cst = sCpool.tile([BH, 512], F32, tag="cst", name="cst")
            nc.scalar.activation(cst[:, 0:w], tt[:, 0:w], TANH,
                                 bias=bc_sb)
            # combine on the otherwise-idle Pool engine; tt holds h-c
            nc.gpsimd.tensor_sub(out=tt[:, 0:w], in0=h_st[:, m0:m0 + w],
                                 in1=cst[:, 0:w])
            nc.gpsimd.tensor_mul(out=tt[:, 0:w], in0=u_st[:, m0:m0 + w],
                                 in1=tt[:, 0:w])
            nc.gpsimd.tensor_add(out=cst[:, 0:w], in0=cst[:, 0:w],
                                 in1=tt[:, 0:w])
            # out DMAs split across rings: trigger-instruction time on one
            # ring otherwise outlasts the compute tail
            nc.sync.dma_start(out=out_d[0][:, m0:m0 + w],
                              in_=cst[0:D_H, 0:w])
            nc.scalar.dma_start(out=out_d[1][:, m0:m0 + w],
                                in_=cst[D_H:BH, 0:w])

        psc2_prev = None
        for g in range(NBAND):
            psc2 = mm_sz(g, zc1T3, "psc2")
            if g > 0:
                cons4(g - 1, psc2_prev)
            psc2_prev = psc2
        cons4(NBAND - 1, psc2_prev)


# ---- host-side driver ----
_CACHED_NC = None
TRACE = False           # set True (e.g. from test.py) to capture an NTFF profile
TRACE_DIR = None
LAST_RESULTS = None     # BassKernelResults of the most recent kernel() call


def _host_prep(x, h, adj, Wf, bf, Wu, bu, Wc, bc):
    """Shard + cast + layout inputs for the 8 cores. Returns list of in_maps."""
    atp = np.zeros((NP, NN), dtype=np.float32)
    atp[:NN] = adj.T * 4096.0
    at8 = atp.astype(E4NP)                       # [4096, 4000]
    blocks = at8.reshape(NCH, CHUNK, NN)
    cols = [np.ascontiguousarray(
        blocks[:, :, BOFF[g]:BOFF[g] + BW[g]].transpose(1, 0, 2)
    ).reshape(CHUNK, NCH * BW[g]) for g in range(NBAND)]
    at_h = np.ascontiguousarray(np.concatenate(cols, axis=1))

    id16 = np.eye(CHUNK, dtype=np.float16)
    id8 = np.eye(CHUNK, dtype=E4NP)

    wsc = {"wf": (1.0, 1 / 32., 1 / 512.), "wu": (1.0, 1 / 32., 1 / 512.),
           "wcx": (1.0, 1 / 32., 1 / 512.), "wcrh": (1.0, 1 / 64., 1 / 512.)}

    wall = np.zeros((C, 12 * D_H), dtype=np.float16)
    for k in range(3):
        wall[:, k * D_H:(k + 1) * D_H] = \
            (Wf[:, k * C:(k + 1) * C].T * wsc["wf"][k]).astype(np.float16)
        wall[:, (3 + k) * D_H:(4 + k) * D_H] = \
            (Wu[:, k * C:(k + 1) * C].T * wsc["wu"][k]).astype(np.float16)
        wall[0:D_IN, (6 + k) * D_H:(7 + k) * D_H] = \
            (Wc[:, k * C:k * C + D_IN].T * wsc["wcx"][k]).astype(np.float16)
        wall[0:D_H, (9 + k) * D_H:(10 + k) * D_H] = \
            (Wc[:, k * C + D_IN:(k + 1) * C].T * wsc["wcrh"][k]
             ).astype(np.float16)

    def bstack(v):
        return np.concatenate([v] * B_LOC).astype(np.float32)

    b3 = np.stack([bstack(bf), bstack(bu), bstack(bc)], axis=1)

    shared = {
        "at": at_h, "id16": id16, "id8": id8, "wall": wall, "b3": b3,
    }
    in_maps = []
    for core in range(NCORES):
        bs = slice(core * B_LOC, (core + 1) * B_LOC)
        z = np.concatenate([x[bs], h[bs]], axis=1)       # [B_LOC, C, NN]
        znm = z.transpose(2, 0, 1)                       # [NN, B_LOC, C]
        ztp = np.zeros((NP, BC), dtype=np.float32)
        ztp[:NN] = znm.reshape(NN, BC)
        zt8 = np.ascontiguousarray(
            ztp.astype(E4NP).reshape(NCH, CHUNK, BC).transpose(1, 0, 2)
        ).reshape(CHUNK, NCH * BC)
        znp = np.zeros((NP, B_LOC, CHUNK), dtype=np.float16)
        znp[:NN, :, :C] = znm
        h_p = np.ascontiguousarray(
            h[bs].astype(np.float16).reshape(BH, NN))
        in_maps.append(dict(shared, zt=zt8, zn=znp, h=h_p))
    return in_maps


def kernel(**inputs):
    global _CACHED_NC, LAST_RESULTS
    inputs = {k: np.asarray(v) for k, v in inputs.items()}
    if _CACHED_NC is None:
        _CACHED_NC = build_program()
    in_maps = _host_prep(**inputs)
    kw = {}
    if TRACE:
        kw = dict(trace=True, tmpdir=TRACE_DIR)
    res = run_bass_kernel_spmd(_CACHED_NC, in_maps,
                               core_ids=list(range(NCORES)), **kw)
    LAST_RESULTS = res
    outs = [res.results[i]["out"] for i in range(NCORES)]
    return np.concatenate(outs, axis=0).astype(np.float32)


if __name__ == "__main__":
    rng = np.random.default_rng(0)
    ins = {
        "x": rng.standard_normal((B, D_IN, NN), dtype=np.float32),
        "h": rng.standard_normal((B, D_H, NN), dtype=np.float32),
        "adj": rng.random((NN, NN), dtype=np.float32) / NN,
        "Wf": rng.standard_normal((D_H, 3 * C), dtype=np.float32) * 0.05,
        "Wu": rng.standard_normal((D_H, 3 * C), dtype=np.float32) * 0.05,
        "Wc": rng.standard_normal((D_H, 3 * C), dtype=np.float32) * 0.05,
        "bf": rng.standard_normal(D_H).astype(np.float32) * 0.05,
        "bu": rng.standard_normal(D_H).astype(np.float32) * 0.05,
        "bc": rng.standard_normal(D_H).astype(np.float32) * 0.05,
    }
    out = kernel(**ins)
    print(out.shape, out.dtype)
